# revision 1
# baseline (speedup 1.0000x reference)
"""Two-layer GCN (PyG GCNConv semantics) on 8 Trainium2 NeuronCores.

Math: out = Ahat @ relu(Ahat @ (X@W1) + b1) @ W2 + b2, with
Ahat = D^-1/2 (A + I) D^-1/2.  The edge normalization factors as
dinv[src]*dinv[dst], so per layer we:
  - pre-scale the source table rows by dinv (folded into PSUM eviction),
  - aggregate with a 0/1-times-dinv_dst one-hot matrix per 128-edge tile
    (segmented matmul on the PE, edges sorted by dst),
  - gather source rows from HBM by index via gpsimd dma_gather (int16
    indices, so the 50000-row table is addressed as two 25000-row halves).

Sharding: destination nodes are split across the 8 cores (6250 each).
Layer-1 dense matmul X@W1 is replicated on every core (cheaper than an
all-gather of the table).  One AllGather shares the layer-2 source table.
"""

import sys

import numpy as np

try:
    import concourse.bass as bass  # noqa: F401
except ImportError:
    sys.path.insert(0, "/opt/trn_rl_repo")

from contextlib import ExitStack

import ml_dtypes

import concourse.bass as bass
import concourse.tile as tile
from concourse import bacc, mybir
from concourse.bass_utils import run_bass_kernel_spmd

BF16 = ml_dtypes.bfloat16

# debug ablation: 0 = no dma_gather + no collective, 1 = gather + no collective,
# 2 = full kernel
ABLATE = 2

N = 50000
E = 800000
FIN = 128
HID = 128
FOUT = 64
NCORES = 8
NSH = N // NCORES  # 6250 destination nodes per core
BLK = 128  # dst block (psum window)
NBLK = (NSH + BLK - 1) // BLK  # 49
SBW = 4  # dst blocks per superblock (one 512-wide psum bank)
NSB = (NBLK + SBW - 1) // SBW  # 13
HALF = 25000  # table half split (int16 gather indices)
NPAD = ((N + 127) // 128) * 128  # 50048
NDTILES = NPAD // 128  # 391


def _layout(tiles):
    """Static program layout from per-(block,half) tile counts.

    Returns (TT, tile_base[NBLK][2], seg: {(sb,h): (tile0, ntiles)}).
    Data/program order: for sb, for half, for block in sb, k tiles.
    """
    tile_base = np.zeros((NBLK, 2), dtype=np.int64)
    seg = {}
    pos = 0
    for sb in range(NSB):
        blocks = range(sb * SBW, min((sb + 1) * SBW, NBLK))
        for h in (0, 1):
            seg_start = pos
            for b in blocks:
                tile_base[b][h] = pos
                pos += int(tiles[b][h])
            seg[(sb, h)] = (seg_start, pos - seg_start)
    return int(pos), tile_base, seg


def _prep(edge_index):
    src = np.asarray(edge_index[0], dtype=np.int64)
    dst = np.asarray(edge_index[1], dtype=np.int64)
    deg = (np.bincount(dst, minlength=N) + 1).astype(np.float64)
    dinv = (1.0 / np.sqrt(deg)).astype(np.float32)

    s_all = np.concatenate([src, np.arange(N, dtype=np.int64)])
    d_all = np.concatenate([dst, np.arange(N, dtype=np.int64)])
    core = d_all // NSH
    local = d_all % NSH
    block = local // BLK
    sbk = block // SBW
    half = (s_all >= HALF).astype(np.int64)

    cidx = (core * NBLK + block) * 2 + half
    cnt = np.bincount(cidx, minlength=NCORES * NBLK * 2).reshape(NCORES, NBLK, 2)
    tiles = ((cnt + BLK - 1) // BLK).max(axis=0)  # [NBLK, 2] max over cores

    TT, tile_base, seg = _layout(tiles)
    S = TT * BLK

    # sort edges into (core, sb, half, block) segment order
    order = np.lexsort((local, block, half, sbk, core))
    s_s = s_all[order]
    d_s = d_all[order]
    core_s = core[order]
    block_s = block[order]
    half_s = half[order]

    gid = (core_s * NBLK + block_s) * 2 + half_s
    change = np.r_[True, gid[1:] != gid[:-1]]
    gstart = np.maximum.accumulate(np.where(change, np.arange(len(gid)), 0))
    rank = np.arange(len(gid)) - gstart
    slot = tile_base[block_s, half_s] * BLK + rank  # per-core slot in [0, S)

    src_loc = np.where(half_s == 0, s_s, s_s - HALF).astype(np.int16)
    dst_loc = (d_s % NSH - block_s * BLK).astype(np.float32)  # 0..127
    dinv_d = dinv[d_s] * dinv[s_s]  # full edge norm dinv_src*dinv_dst

    seg_slot0 = np.zeros((NSB, 2), dtype=np.int64)
    for (sb, h), (t0, _nt) in seg.items():
        seg_slot0[sb, h] = t0 * BLK

    meta_np = np.zeros((NCORES, 128, TT, 2), dtype=np.float32)
    idx_np = np.zeros((NCORES, 128, S // 16), dtype=np.int16)
    for c in range(NCORES):
        m = core_s == c
        sl = slot[m]
        tt = sl // BLK
        pp = sl % BLK
        meta_np[c, pp, tt, 0] = dst_loc[m]
        meta_np[c, pp, tt, 1] = dinv_d[m]
        seg0 = seg_slot0[block_s[m] // SBW, half_s[m]]
        j = sl - seg0
        col = seg0 // 16 + j // 16
        row = j % 16
        v = src_loc[m]
        for g in range(8):  # replicate across the 8 gpsimd 16-partition groups
            idx_np[c, row + 16 * g, col] = v

    dinv_blk = np.zeros((NCORES, 128, NBLK), dtype=np.float32)
    ids = np.arange(NBLK * 128)
    valid = ids < NSH
    for c in range(NCORES):
        tmp = np.zeros(NBLK * 128, np.float32)
        tmp[valid] = dinv[c * NSH + ids[valid]]
        dinv_blk[c] = tmp.reshape(NBLK, 128).T

    tmp2 = np.zeros(NPAD, np.float32)
    tmp2[:N] = dinv
    dinv_dense = np.ascontiguousarray(tmp2.reshape(NDTILES, 128).T)  # [128, NDTILES]

    return tiles, dinv, meta_np, idx_np, dinv_blk, dinv_dense


def _build(tiles):
    """Build the (single, SPMD) Bacc program for the given tile counts."""
    TT, tile_base, seg = _layout(tiles)
    S = TT * BLK
    f32 = mybir.dt.float32
    bf16 = mybir.dt.bfloat16
    i16 = mybir.dt.int16
    AF = mybir.ActivationFunctionType
    OP = mybir.AluOpType

    nc = bacc.Bacc("TRN2", target_bir_lowering=False, debug=False, num_devices=NCORES)
    xT = nc.dram_tensor("xT", [128, NPAD], bf16, kind="ExternalInput")
    w1 = nc.dram_tensor("w1", [128, HID], bf16, kind="ExternalInput")
    w2 = nc.dram_tensor("w2", [128, FOUT], bf16, kind="ExternalInput")
    b1r = nc.dram_tensor("b1r", [128, HID], f32, kind="ExternalInput")
    b2r = nc.dram_tensor("b2r", [128, FOUT], f32, kind="ExternalInput")
    iot = nc.dram_tensor("iot", [128, BLK], bf16, kind="ExternalInput")
    meta = nc.dram_tensor("meta", [128, TT, 2], f32, kind="ExternalInput")
    idxt = nc.dram_tensor("idx", [128, S // 16], i16, kind="ExternalInput")
    outp = nc.dram_tensor("out", [NSH, FOUT], f32, kind="ExternalOutput")

    with tile.TileContext(nc) as tc, ExitStack() as ctx:
        const = ctx.enter_context(tc.tile_pool(name="const", bufs=1))
        dram = ctx.enter_context(tc.tile_pool(name="dram", bufs=1, space="DRAM"))
        xin = ctx.enter_context(tc.tile_pool(name="xin", bufs=4))
        t1ev = ctx.enter_context(tc.tile_pool(name="t1ev", bufs=4))
        gpool = ctx.enter_context(tc.tile_pool(name="g", bufs=3))
        tpp = ctx.enter_context(tc.tile_pool(name="tp", bufs=12))
        evp = ctx.enter_context(tc.tile_pool(name="ev", bufs=4))
        psd = ctx.enter_context(tc.tile_pool(name="psd", bufs=2, space="PSUM"))
        psa = ctx.enter_context(tc.tile_pool(name="psa", bufs=4, space="PSUM"))
        pso = ctx.enter_context(tc.tile_pool(name="pso", bufs=2, space="PSUM"))

        def cload(ap, shape, dtype, tag):
            t = const.tile(shape, dtype, tag=tag)
            nc.sync.dma_start(t[:], ap)
            return t

        w1_sb = cload(w1[:, :], [128, HID], bf16, "w1")
        w2_sb = cload(w2[:, :], [128, FOUT], bf16, "w2")
        b1_sb = cload(b1r[:, :], [128, HID], f32, "b1")
        b2_sb = cload(b2r[:, :], [128, FOUT], f32, "b2")
        iota_sb = cload(iot[:, :], [128, BLK], bf16, "iota")
        meta_sb = cload(meta[:, :, :], [128, TT, 2], f32, "meta")
        idx_sb = cload(idxt[:, :], [128, S // 16], i16, "idx")

        table1 = dram.tile([NPAD, HID], bf16, tag="table1")
        t2loc = dram.tile([NSH, HID], bf16, tag="t2loc")
        t2full = dram.tile([N, HID], bf16, tag="t2full")

        # Phase A: table1 = X @ W1, replicated on every core (norm lives in meta)
        DB = 4  # node tiles per DMA batch
        for j0 in range(0, NDTILES, DB):
            nb = min(DB, NDTILES - j0)
            xt = xin.tile([128, DB * 128], bf16, tag="xt")
            nc.sync.dma_start(
                xt[:, 0 : nb * 128], xT[:, j0 * 128 : (j0 + nb) * 128]
            )
            ev = t1ev.tile([128, DB, HID], bf16, tag="t1ev")
            for i in range(nb):
                ps = psd.tile([128, HID], f32, tag="psd")
                nc.tensor.matmul(
                    ps[:],
                    lhsT=xt[:, i * 128 : (i + 1) * 128],
                    rhs=w1_sb[:],
                    start=True,
                    stop=True,
                )
                nc.scalar.activation(ev[:, i, :], ps[:], AF.Copy)
            # table1 row j*128+p <- ev[p, j-j0, :]
            nc.sync.dma_start(
                table1[j0 * 128 : (j0 + nb) * 128, :].rearrange(
                    "(t p) f -> p t f", p=128
                ),
                ev[:, 0:nb, :],
            )

        def agg(layer):
            table = table1 if layer == 1 else t2full
            for sb in range(NSB):
                blocks = list(range(sb * SBW, min((sb + 1) * SBW, NBLK)))
                nbl = len(blocks)
                gt = {}
                for h in (0, 1):
                    t0, ntl = seg[(sb, h)]
                    if ntl == 0:
                        continue
                    g = gpool.tile([128, ntl, 128], bf16, tag=f"g{h}")
                    if ABLATE >= 1:
                        view = table[0:HALF, :] if h == 0 else table[HALF : 2 * HALF, :]
                        # SWDGE descriptor ring limit: <= 1024 idx per gather
                        GCH = 8  # tiles per gather chunk
                        for q0 in range(0, ntl, GCH):
                            qn = min(GCH, ntl - q0)
                            c0 = (t0 + q0) * 8  # idx columns (tile*128/16)
                            nc.gpsimd.dma_gather(
                                out_ap=g[:, q0 : q0 + qn, :],
                                in_ap=view,
                                idxs_ap=idx_sb[:, c0 : c0 + qn * 8],
                                num_idxs=qn * 128,
                                num_idxs_reg=qn * 128,
                                elem_size=HID,
                                queue_num=0,
                            )
                    else:
                        nc.vector.memset(g[:], 0)
                    gt[h] = g
                for b in blocks:
                    ps = psa.tile([128, BLK], f32, tag="psa")  # one bank per block
                    for h in (0, 1):
                        if seg[(sb, h)][1] == 0 or tiles[b][h] == 0:
                            continue
                        seg_t0 = seg[(sb, h)][0]
                        for k in range(int(tiles[b][h])):
                            t = int(tile_base[b][h]) + k
                            gofs = t - seg_t0
                            first = (k == 0) and (h == 0 or tiles[b][0] == 0)
                            last = (k == int(tiles[b][h]) - 1) and (
                                h == 1 or tiles[b][1] == 0
                            )
                            tp = tpp.tile([128, BLK], bf16, tag="tp")
                            nc.vector.tensor_scalar(
                                out=tp[:],
                                in0=iota_sb[:],
                                scalar1=meta_sb[:, t, 0:1],
                                scalar2=meta_sb[:, t, 1:2],
                                op0=OP.is_equal,
                                op1=OP.mult,
                            )
                            gtile = gt[h][:, gofs, :]
                            if layer == 1:
                                nc.tensor.matmul(
                                    ps[:], lhsT=tp[:], rhs=gtile, start=first, stop=last
                                )
                            else:
                                nc.tensor.matmul(
                                    ps[:], lhsT=gtile, rhs=tp[:], start=first, stop=last
                                )
                    r0 = b * BLK
                    r1 = min(NSH, r0 + BLK)
                    if layer == 1:
                        s1 = evp.tile([128, HID], f32, tag="s1")
                        nc.vector.tensor_add(s1[:], ps[:], b1_sb[:])
                        ev = evp.tile([128, HID], bf16, tag="t2ev")
                        nc.scalar.activation(ev[:], s1[:], AF.Relu)
                        nc.sync.dma_start(t2loc[r0:r1, :], ev[0 : r1 - r0, :])
                    else:
                        ag2 = evp.tile([128, BLK], bf16, tag="ag2")
                        nc.scalar.activation(ag2[:], ps[:], AF.Copy)
                        po = pso.tile([128, FOUT], f32, tag="pso")
                        nc.tensor.matmul(
                            po[:], lhsT=ag2[:], rhs=w2_sb[:], start=True, stop=True
                        )
                        oo = evp.tile([128, FOUT], f32, tag="oo")
                        nc.vector.tensor_add(oo[:], po[:], b2_sb[:])
                        nc.sync.dma_start(outp[r0:r1, :], oo[0 : r1 - r0, :])

        agg(1)
        if ABLATE >= 2:
            nc.gpsimd.collective_compute(
                "AllGather",
                mybir.AluOpType.bypass,
                replica_groups=[list(range(NCORES))],
                ins=[t2loc[:].opt()],
                outs=[t2full[:].opt()],
            )
        else:
            nc.sync.dma_start(t2full[0:NSH, :], t2loc[:, :])
        agg(2)

    nc.finalize()
    return nc


def _in_maps(x, W1, b1, W2, b2, prep):
    tiles, dinv, meta_np, idx_np, dinv_blk, dinv_dense = prep
    xT = np.zeros((128, NPAD), dtype=BF16)
    xT[:, :N] = np.asarray(x, np.float32).T.astype(BF16)
    w1b = np.asarray(W1, np.float32).astype(BF16)
    w2b = np.asarray(W2, np.float32).astype(BF16)
    b1rep = np.broadcast_to(np.asarray(b1, np.float32), (128, HID)).copy()
    b2rep = np.broadcast_to(np.asarray(b2, np.float32), (128, FOUT)).copy()
    iota = np.broadcast_to(np.arange(BLK, dtype=np.float32), (128, BLK)).astype(BF16)
    shared = {
        "xT": xT,
        "w1": w1b,
        "w2": w2b,
        "b1r": b1rep,
        "b2r": b2rep,
        "iot": np.ascontiguousarray(iota),
    }
    return [
        dict(
            shared,
            meta=np.ascontiguousarray(meta_np[c]),
            idx=np.ascontiguousarray(idx_np[c]),
        )
        for c in range(NCORES)
    ]


def kernel(x, edge_index, W1, b1, W2, b2):
    prep = _prep(edge_index)
    nc = _build(prep[0])
    in_maps = _in_maps(x, W1, b1, W2, b2, prep)
    res = run_bass_kernel_spmd(nc, in_maps, core_ids=list(range(NCORES)), trace=False)
    out = np.concatenate(
        [res.results[c]["out"].astype(np.float32) for c in range(NCORES)], axis=0
    )
    return out



# revision 3
# speedup vs baseline: 1.0543x; 1.0543x over previous
"""Two-layer GCN (PyG GCNConv semantics) on 8 Trainium2 NeuronCores.

Math: out = Ahat @ relu(Ahat @ (X@W1) + b1) @ W2 + b2, with
Ahat = D^-1/2 (A + I) D^-1/2.

Restructured vs the phase-A baseline:
  - Layer 1 aggregates RAW X rows (Ahat@X) gathered straight from the
    input table, then applies W1 per 128-row destination block on-chip
    ((Ahat X) W1 == Ahat (X W1)); no replicated dense pre-pass, no
    table1 spill/reload.
  - The block result is relu'd (bias folded into the activation after a
    PE transpose) and immediately multiplied by W2, so the collective
    exchanges the 64-wide h2 = relu(.)@W2 table (6.4 MB instead of
    12.8 MB).  Aggregation is linear, so Ahat(h)W2 == Ahat(h W2).
  - The h2 table is PAIR-PACKED [25000, 128] bf16 (dma_gather needs
    256B-multiple rows): layer-2 gathers fetch a node pair, and a
    256-wide one-hot (dst + 128*parity) feeds two matmuls that pick the
    correct half.  Pair indices fit int16, so layer 2 needs no halves.

Sharding: destination nodes split across 8 cores (6250 each); one
AllGather (Shared output) shares the layer-2 source table.
"""

import sys

import numpy as np

try:
    import concourse.bass as bass  # noqa: F401
except ImportError:
    sys.path.insert(0, "/opt/trn_rl_repo")

from contextlib import ExitStack

import ml_dtypes

import concourse.bass as bass
import concourse.tile as tile
from concourse import bacc, mybir
from concourse.bass_utils import run_bass_kernel_spmd

BF16 = ml_dtypes.bfloat16

N = 50000
E = 800000
FIN = 128
HID = 128
FOUT = 64
NCORES = 8
NSH = N // NCORES  # 6250 destination nodes per core
BLK = 128  # dst block (psum window)
NBLK = (NSH + BLK - 1) // BLK  # 49
SBW = 4  # dst blocks per superblock (layer-1 gather segmenting)
NSB = (NBLK + SBW - 1) // SBW  # 13
HALF = 25000  # layer-1 table half split (int16 gather indices)
NPAIR = 25000  # layer-2 pair-packed table rows
GCH = 8  # tiles per dma_gather chunk (8*128 = 1024 = SWDGE ring)


def _layout1(tiles):
    """Layer-1 static layout from per-(block,half) tile counts.

    Returns (TT, tile_base[NBLK][2], seg: {(sb,h): (tile0, ntiles)}).
    Data/program order: for sb, for half, for block in sb, k tiles.
    """
    tile_base = np.zeros((NBLK, 2), dtype=np.int64)
    seg = {}
    pos = 0
    for sb in range(NSB):
        blocks = range(sb * SBW, min((sb + 1) * SBW, NBLK))
        for h in (0, 1):
            seg_start = pos
            for b in blocks:
                tile_base[b][h] = pos
                pos += int(tiles[b][h])
            seg[(sb, h)] = (seg_start, pos - seg_start)
    return int(pos), tile_base, seg


def _rank_within_groups(gid):
    change = np.r_[True, gid[1:] != gid[:-1]]
    gstart = np.maximum.accumulate(np.where(change, np.arange(len(gid)), 0))
    return np.arange(len(gid)) - gstart


def _fill_meta_idx(core_s, slot, dval, nval, srcv, TT, S):
    """Build per-core meta [128, TT, 2] f32 and idx [128, S//16] i16 tables."""
    meta_np = np.zeros((NCORES, 128, TT, 2), dtype=np.float32)
    idx_np = np.zeros((NCORES, 128, S // 16), dtype=np.int16)
    for c in range(NCORES):
        m = core_s == c
        sl = slot[m]
        tt = sl // BLK
        pp = sl % BLK
        meta_np[c, pp, tt, 0] = dval[m]
        meta_np[c, pp, tt, 1] = nval[m]
        col = sl // 16
        row = sl % 16
        v = srcv[m]
        for g in range(8):  # replicate across the 8 gpsimd 16-partition groups
            idx_np[c, row + 16 * g, col] = v
    return meta_np, idx_np


def _prep(edge_index):
    src = np.asarray(edge_index[0], dtype=np.int64)
    dst = np.asarray(edge_index[1], dtype=np.int64)
    deg = (np.bincount(dst, minlength=N) + 1).astype(np.float64)
    dinv = (1.0 / np.sqrt(deg)).astype(np.float32)

    s_all = np.concatenate([src, np.arange(N, dtype=np.int64)])
    d_all = np.concatenate([dst, np.arange(N, dtype=np.int64)])
    norm_all = dinv[s_all] * dinv[d_all]
    core = d_all // NSH
    local = d_all % NSH
    block = local // BLK
    dstloc = (local % BLK).astype(np.float32)

    # ---- layer 1: halves (int16 src index into x halves), sb segments ----
    half = (s_all >= HALF).astype(np.int64)
    sbk = block // SBW
    cidx = (core * NBLK + block) * 2 + half
    cnt = np.bincount(cidx, minlength=NCORES * NBLK * 2).reshape(NCORES, NBLK, 2)
    tiles1 = ((cnt + BLK - 1) // BLK).max(axis=0)  # [NBLK, 2]
    TT1, tbase1, seg1 = _layout1(tiles1)
    S1 = TT1 * BLK

    order = np.lexsort((local, block, half, sbk, core))
    s_s = s_all[order]
    core_s = core[order]
    block_s = block[order]
    half_s = half[order]
    gid = (core_s * NBLK + block_s) * 2 + half_s
    rank = _rank_within_groups(gid)
    slot1 = tbase1[block_s, half_s] * BLK + rank
    src_loc = np.where(half_s == 0, s_s, s_s - HALF).astype(np.int16)
    meta1_np, idx1_np = _fill_meta_idx(
        core_s, slot1, dstloc[order], norm_all[order], src_loc, TT1, S1
    )

    # ---- layer 2: pair-packed table, block-major, no halves ----
    cidx2 = core * NBLK + block
    cnt2 = np.bincount(cidx2, minlength=NCORES * NBLK).reshape(NCORES, NBLK)
    tiles2 = ((cnt2 + BLK - 1) // BLK).max(axis=0)  # [NBLK]
    tbase2 = np.concatenate([[0], np.cumsum(tiles2)[:-1]]).astype(np.int64)
    TT2 = int(tiles2.sum())
    S2 = TT2 * BLK

    order2 = np.lexsort((local, block, core))
    s2 = s_all[order2]
    core2 = core[order2]
    block2 = block[order2]
    gid2 = core2 * NBLK + block2
    rank2 = _rank_within_groups(gid2)
    slot2 = tbase2[block2] * BLK + rank2
    srcp = (s2 >> 1).astype(np.int16)
    dadj = dstloc[order2] + 128.0 * (s2 & 1)
    meta2_np, idx2_np = _fill_meta_idx(
        core2, slot2, dadj, norm_all[order2], srcp, TT2, S2
    )

    layout = (tiles1, tiles2)
    return layout, meta1_np, idx1_np, meta2_np, idx2_np


def _build(layout):
    """Build the (single, SPMD) Bacc program for the given tile counts."""
    tiles1, tiles2 = layout
    TT1, tbase1, seg1 = _layout1(tiles1)
    S1 = TT1 * BLK
    tbase2 = np.concatenate([[0], np.cumsum(tiles2)[:-1]]).astype(np.int64)
    TT2 = int(tiles2.sum())
    S2 = TT2 * BLK
    f32 = mybir.dt.float32
    bf16 = mybir.dt.bfloat16
    i16 = mybir.dt.int16
    AF = mybir.ActivationFunctionType
    OP = mybir.AluOpType

    nc = bacc.Bacc("TRN2", target_bir_lowering=False, debug=False, num_devices=NCORES)
    xin = nc.dram_tensor("xin", [N, FIN], bf16, kind="ExternalInput")
    w1 = nc.dram_tensor("w1", [FIN, HID], bf16, kind="ExternalInput")
    w2 = nc.dram_tensor("w2", [HID, FOUT], bf16, kind="ExternalInput")
    b1c = nc.dram_tensor("b1c", [HID, 1], f32, kind="ExternalInput")
    b2r = nc.dram_tensor("b2r", [128, FOUT], f32, kind="ExternalInput")
    iot = nc.dram_tensor("iot", [128, BLK], bf16, kind="ExternalInput")
    io2 = nc.dram_tensor("io2", [128, 2 * BLK], bf16, kind="ExternalInput")
    idn = nc.dram_tensor("idn", [128, 128], bf16, kind="ExternalInput")
    meta1 = nc.dram_tensor("meta1", [128, TT1, 2], f32, kind="ExternalInput")
    idx1t = nc.dram_tensor("idx1", [128, S1 // 16], i16, kind="ExternalInput")
    meta2 = nc.dram_tensor("meta2", [128, TT2, 2], f32, kind="ExternalInput")
    idx2t = nc.dram_tensor("idx2", [128, S2 // 16], i16, kind="ExternalInput")
    outp = nc.dram_tensor("out", [NSH, FOUT], f32, kind="ExternalOutput")

    with tile.TileContext(nc) as tc, ExitStack() as ctx:
        const = ctx.enter_context(tc.tile_pool(name="const", bufs=1))
        dram = ctx.enter_context(tc.tile_pool(name="dram", bufs=1, space="DRAM"))
        gpool = ctx.enter_context(tc.tile_pool(name="g", bufs=3))
        g2pool = ctx.enter_context(tc.tile_pool(name="g2", bufs=3))
        tpp = ctx.enter_context(tc.tile_pool(name="tp", bufs=12))
        evp = ctx.enter_context(tc.tile_pool(name="ev", bufs=4))
        psa = ctx.enter_context(tc.tile_pool(name="psa", bufs=3, space="PSUM"))
        psd = ctx.enter_context(tc.tile_pool(name="psd", bufs=1, space="PSUM"))
        pst = ctx.enter_context(tc.tile_pool(name="pst", bufs=1, space="PSUM"))
        pso = ctx.enter_context(tc.tile_pool(name="pso", bufs=1, space="PSUM"))
        psb = ctx.enter_context(tc.tile_pool(name="psb", bufs=2, space="PSUM"))

        def cload(ap, shape, dtype, tag):
            t = const.tile(shape, dtype, tag=tag)
            nc.sync.dma_start(t[:], ap)
            return t

        w1_sb = cload(w1[:, :], [FIN, HID], bf16, "w1")
        w2_sb = cload(w2[:, :], [HID, FOUT], bf16, "w2")
        b1_sb = cload(b1c[:, :], [HID, 1], f32, "b1")
        b2_sb = cload(b2r[:, :], [128, FOUT], f32, "b2")
        iota_sb = cload(iot[:, :], [128, BLK], bf16, "iota")
        io2_sb = cload(io2[:, :], [128, 2 * BLK], bf16, "io2")
        idn_sb = cload(idn[:, :], [128, 128], bf16, "idn")
        meta1_sb = cload(meta1[:, :, :], [128, TT1, 2], f32, "meta1")
        idx1_sb = cload(idx1t[:, :], [128, S1 // 16], i16, "idx1")
        meta2_sb = cload(meta2[:, :, :], [128, TT2, 2], f32, "meta2")
        idx2_sb = cload(idx2t[:, :], [128, S2 // 16], i16, "idx2")

        t2loc = dram.tile([NSH, FOUT], bf16, tag="t2loc")
        t2full = dram.tile([N, FOUT], bf16, tag="t2full", addr_space="Shared")

        def finalize1(b, ps):
            """ps [x=128, d=128] f32 -> h2 = relu((Ahat X W1)+b1) @ W2 -> t2loc."""
            ag = evp.tile([128, BLK], bf16, tag="ag")
            nc.scalar.activation(ag[:], ps[:], AF.Copy)
            hp = psd.tile([128, HID], f32, tag="hp")
            nc.tensor.matmul(hp[:], lhsT=ag[:], rhs=w1_sb[:], start=True, stop=True)
            hpe = evp.tile([128, HID], bf16, tag="hpe")
            nc.scalar.activation(hpe[:], hp[:], AF.Copy)
            pT = pst.tile([128, BLK], bf16, tag="pT")
            nc.tensor.transpose(pT[:], hpe[:], idn_sb[:])
            ev = evp.tile([128, BLK], bf16, tag="evt")
            nc.scalar.activation(ev[:], pT[:], AF.Relu, bias=b1_sb[:, 0:1])
            h2p = pso.tile([128, FOUT], f32, tag="h2p")
            nc.tensor.matmul(h2p[:], lhsT=ev[:], rhs=w2_sb[:], start=True, stop=True)
            h2 = evp.tile([128, FOUT], bf16, tag="h2")
            nc.scalar.activation(h2[:], h2p[:], AF.Copy)
            r0 = b * BLK
            r1 = min(NSH, r0 + BLK)
            nc.sync.dma_start(t2loc[r0:r1, :], h2[0 : r1 - r0, :])

        # ---- layer 1: aggregate raw X rows, then per-block W1/relu/W2 ----
        for sb in range(NSB):
            blocks = list(range(sb * SBW, min((sb + 1) * SBW, NBLK)))
            gt = {}
            for h in (0, 1):
                t0, ntl = seg1[(sb, h)]
                if ntl == 0:
                    continue
                g = gpool.tile([128, ntl, FIN], bf16, tag=f"g{h}")
                view = xin[0:HALF, :] if h == 0 else xin[HALF : 2 * HALF, :]
                for q0 in range(0, ntl, GCH):
                    qn = min(GCH, ntl - q0)
                    c0 = (t0 + q0) * 8  # idx columns (tile*128/16)
                    nc.gpsimd.dma_gather(
                        out_ap=g[:, q0 : q0 + qn, :],
                        in_ap=view,
                        idxs_ap=idx1_sb[:, c0 : c0 + qn * 8],
                        num_idxs=qn * 128,
                        num_idxs_reg=qn * 128,
                        elem_size=FIN,
                        queue_num=0,
                    )
                gt[h] = (g, t0)
            for b in blocks:
                ps = psa.tile([128, BLK], f32, tag="psa")
                for h in (0, 1):
                    if seg1[(sb, h)][1] == 0 or tiles1[b][h] == 0:
                        continue
                    g, seg_t0 = gt[h]
                    for k in range(int(tiles1[b][h])):
                        t = int(tbase1[b][h]) + k
                        first = (k == 0) and (h == 0 or tiles1[b][0] == 0)
                        last = (k == int(tiles1[b][h]) - 1) and (
                            h == 1 or tiles1[b][1] == 0
                        )
                        tp = tpp.tile([128, BLK], bf16, tag="tp")
                        nc.vector.tensor_scalar(
                            out=tp[:],
                            in0=iota_sb[:],
                            scalar1=meta1_sb[:, t, 0:1],
                            scalar2=meta1_sb[:, t, 1:2],
                            op0=OP.is_equal,
                            op1=OP.mult,
                        )
                        nc.tensor.matmul(
                            ps[:],
                            lhsT=g[:, t - seg_t0, :],
                            rhs=tp[:],
                            start=first,
                            stop=last,
                        )
                finalize1(b, ps)

        # ---- exchange the pair-packed h2 table ----
        nc.gpsimd.collective_compute(
            "AllGather",
            mybir.AluOpType.bypass,
            replica_groups=[list(range(NCORES))],
            ins=[t2loc[:].opt()],
            outs=[t2full[:].opt()],
        )
        t2pair = t2full[:, :].rearrange("(a b) c -> a (b c)", b=2)  # [25000, 128]

        # ---- layer 2: gather node pairs, 256-wide one-hot, two matmuls ----
        block_of_tile2 = np.repeat(np.arange(NBLK), tiles2)
        g2 = None
        ps2 = None
        for t in range(TT2):
            if t % GCH == 0:
                qn = min(GCH, TT2 - t)
                g2 = g2pool.tile([128, GCH, FIN], bf16, tag="g2")
                nc.gpsimd.dma_gather(
                    out_ap=g2[:, 0:qn, :],
                    in_ap=t2pair,
                    idxs_ap=idx2_sb[:, t * 8 : (t + qn) * 8],
                    num_idxs=qn * 128,
                    num_idxs_reg=qn * 128,
                    elem_size=FIN,
                    queue_num=0,
                )
            b = int(block_of_tile2[t])
            k = t - int(tbase2[b])
            first = k == 0
            last = k == int(tiles2[b]) - 1
            if first:
                ps2 = psb.tile([128, FOUT], f32, tag="psb")
            tp2 = tpp.tile([128, 2 * BLK], bf16, tag="tp2")
            nc.vector.tensor_scalar(
                out=tp2[:],
                in0=io2_sb[:],
                scalar1=meta2_sb[:, t, 0:1],
                scalar2=meta2_sb[:, t, 1:2],
                op0=OP.is_equal,
                op1=OP.mult,
            )
            j = t % GCH
            nc.tensor.matmul(
                ps2[:],
                lhsT=tp2[:, 0:BLK],
                rhs=g2[:, j, 0:FOUT],
                start=first,
                stop=False,
            )
            nc.tensor.matmul(
                ps2[:],
                lhsT=tp2[:, BLK : 2 * BLK],
                rhs=g2[:, j, FOUT:FIN],
                start=False,
                stop=last,
            )
            if last:
                oo = evp.tile([128, FOUT], f32, tag="oo")
                nc.vector.tensor_add(oo[:], ps2[:], b2_sb[:])
                r0 = b * BLK
                r1 = min(NSH, r0 + BLK)
                nc.sync.dma_start(outp[r0:r1, :], oo[0 : r1 - r0, :])

    nc.finalize()
    return nc


def _in_maps(x, W1, b1, W2, b2, prep):
    layout, meta1_np, idx1_np, meta2_np, idx2_np = prep
    xb = np.asarray(x, np.float32).astype(BF16)
    w1b = np.asarray(W1, np.float32).astype(BF16)
    w2b = np.asarray(W2, np.float32).astype(BF16)
    b1col = np.asarray(b1, np.float32).reshape(HID, 1).copy()
    b2rep = np.broadcast_to(np.asarray(b2, np.float32), (128, FOUT)).copy()
    iota = np.broadcast_to(np.arange(BLK, dtype=np.float32), (128, BLK)).astype(BF16)
    io2 = np.broadcast_to(np.arange(2 * BLK, dtype=np.float32), (128, 2 * BLK)).astype(
        BF16
    )
    idn = np.eye(128, dtype=np.float32).astype(BF16)
    shared = {
        "xin": np.ascontiguousarray(xb),
        "w1": w1b,
        "w2": w2b,
        "b1c": b1col,
        "b2r": b2rep,
        "iot": np.ascontiguousarray(iota),
        "io2": np.ascontiguousarray(io2),
        "idn": idn,
    }
    return [
        dict(
            shared,
            meta1=np.ascontiguousarray(meta1_np[c]),
            idx1=np.ascontiguousarray(idx1_np[c]),
            meta2=np.ascontiguousarray(meta2_np[c]),
            idx2=np.ascontiguousarray(idx2_np[c]),
        )
        for c in range(NCORES)
    ]


def kernel(x, edge_index, W1, b1, W2, b2):
    prep = _prep(edge_index)
    nc = _build(prep[0])
    in_maps = _in_maps(x, W1, b1, W2, b2, prep)
    res = run_bass_kernel_spmd(nc, in_maps, core_ids=list(range(NCORES)), trace=False)
    out = np.concatenate(
        [res.results[c]["out"].astype(np.float32) for c in range(NCORES)], axis=0
    )
    return out


# revision 14
# speedup vs baseline: 1.4530x; 1.3781x over previous
"""Two-layer GCN (PyG GCNConv semantics) on 8 Trainium2 NeuronCores.

Math: out = Ahat @ relu(Ahat @ (X@W1) + b1) @ W2 + b2, with
Ahat = D^-1/2 (A + I) D^-1/2.

Restructured vs the phase-A baseline:
  - Layer 1 aggregates RAW X rows (Ahat@X) gathered straight from the
    input table, then applies W1 per 128-row destination block on-chip
    ((Ahat X) W1 == Ahat (X W1)); no replicated dense pre-pass, no
    table1 spill/reload.
  - The block result is relu'd (bias folded into the activation after a
    PE transpose) and immediately multiplied by W2, so the collective
    exchanges the 64-wide h2 = relu(.)@W2 table (6.4 MB instead of
    12.8 MB).  Aggregation is linear, so Ahat(h)W2 == Ahat(h W2).
  - The h2 table is PAIR-PACKED [25000, 128] bf16 (dma_gather needs
    256B-multiple rows): layer-2 gathers fetch a node pair, and a
    256-wide one-hot (dst + 128*parity) feeds two matmuls that pick the
    correct half.  Pair indices fit int16, so layer 2 needs no halves.

Sharding: destination nodes split across 8 cores (6250 each); one
AllGather (Shared output) shares the layer-2 source table.
"""

import sys

import numpy as np

try:
    import concourse.bass as bass  # noqa: F401
except ImportError:
    sys.path.insert(0, "/opt/trn_rl_repo")

from contextlib import ExitStack

import ml_dtypes

import concourse.bass as bass
import concourse.tile as tile
from concourse import bacc, mybir
from concourse.bass_utils import run_bass_kernel_spmd

BF16 = ml_dtypes.bfloat16

N = 50000
E = 800000
FIN = 128
HID = 128
FOUT = 64
NCORES = 8
NSH = N // NCORES  # 6250 destination nodes per core
BLK = 128  # dst block (psum window)
NBLK = (NSH + BLK - 1) // BLK  # 49
SBW = 4  # dst blocks per superblock (layer-1 gather segmenting)
NSB = (NBLK + SBW - 1) // SBW  # 13
HALF = 25000  # layer-1 table half split (int16 gather indices)
NPAIR = 25000  # layer-2 pair-packed table rows
GCH = 8  # tiles per dma_gather chunk (8*128 = 1024 = SWDGE ring)


def _layout1(tiles):
    """Layer-1 static layout from per-(block,half) tile counts.

    Returns (TT, tile_base[NBLK][2], seg: {(sb,h): (tile0, ntiles)}).
    Data/program order: for sb, for half, for block in sb, k tiles.
    """
    tile_base = np.zeros((NBLK, 2), dtype=np.int64)
    seg = {}
    pos = 0
    for sb in range(NSB):
        blocks = range(sb * SBW, min((sb + 1) * SBW, NBLK))
        for h in (0, 1):
            seg_start = pos
            for b in blocks:
                tile_base[b][h] = pos
                pos += int(tiles[b][h])
            seg[(sb, h)] = (seg_start, pos - seg_start)
    return int(pos), tile_base, seg


def _rank_within_groups(gid):
    change = np.r_[True, gid[1:] != gid[:-1]]
    gstart = np.maximum.accumulate(np.where(change, np.arange(len(gid)), 0))
    return np.arange(len(gid)) - gstart


def _fill_meta_idx(core_s, slot, dval, nval, srcv, TT, S):
    """Build per-core meta [128, TT, 2] f32 and idx [128, S//16] i16 tables."""
    meta_np = np.zeros((NCORES, 128, TT, 2), dtype=np.float32)
    idx_np = np.zeros((NCORES, 128, S // 16), dtype=np.int16)
    for c in range(NCORES):
        m = core_s == c
        sl = slot[m]
        tt = sl // BLK
        pp = sl % BLK
        meta_np[c, pp, tt, 0] = dval[m]
        meta_np[c, pp, tt, 1] = nval[m]
        col = sl // 16
        row = sl % 16
        v = srcv[m]
        for g in range(8):  # replicate across the 8 gpsimd 16-partition groups
            idx_np[c, row + 16 * g, col] = v
    return meta_np, idx_np


def _prep(edge_index):
    src = np.asarray(edge_index[0], dtype=np.int64)
    dst = np.asarray(edge_index[1], dtype=np.int64)
    deg = (np.bincount(dst, minlength=N) + 1).astype(np.float64)
    dinv = (1.0 / np.sqrt(deg)).astype(np.float32)

    s_all = np.concatenate([src, np.arange(N, dtype=np.int64)])
    d_all = np.concatenate([dst, np.arange(N, dtype=np.int64)])
    norm_all = dinv[s_all] * dinv[d_all]
    core = d_all // NSH
    local = d_all % NSH
    block = local // BLK
    dstloc = (local % BLK).astype(np.float32)

    # ---- layer 1: halves (int16 src index into x halves), sb segments ----
    half = (s_all >= HALF).astype(np.int64)
    sbk = block // SBW
    cidx = (core * NBLK + block) * 2 + half
    cnt = np.bincount(cidx, minlength=NCORES * NBLK * 2).reshape(NCORES, NBLK, 2)
    tiles1 = ((cnt + BLK - 1) // BLK).max(axis=0)  # [NBLK, 2]
    TT1, tbase1, seg1 = _layout1(tiles1)
    S1 = TT1 * BLK

    order = np.lexsort((local, block, half, sbk, core))
    s_s = s_all[order]
    core_s = core[order]
    block_s = block[order]
    half_s = half[order]
    gid = (core_s * NBLK + block_s) * 2 + half_s
    rank = _rank_within_groups(gid)
    slot1 = tbase1[block_s, half_s] * BLK + rank
    src_loc = np.where(half_s == 0, s_s, s_s - HALF).astype(np.int16)
    meta1_np, idx1_np = _fill_meta_idx(
        core_s, slot1, dstloc[order], norm_all[order], src_loc, TT1, S1
    )

    # ---- layer 2: pair-packed table, block-major, no halves ----
    cidx2 = core * NBLK + block
    cnt2 = np.bincount(cidx2, minlength=NCORES * NBLK).reshape(NCORES, NBLK)
    tiles2 = ((cnt2 + BLK - 1) // BLK).max(axis=0)  # [NBLK]
    tbase2 = np.concatenate([[0], np.cumsum(tiles2)[:-1]]).astype(np.int64)
    TT2 = int(tiles2.sum())
    S2 = TT2 * BLK

    order2 = np.lexsort((local, block, core))
    s2 = s_all[order2]
    core2 = core[order2]
    block2 = block[order2]
    gid2 = core2 * NBLK + block2
    rank2 = _rank_within_groups(gid2)
    slot2 = tbase2[block2] * BLK + rank2
    srcp = (s2 >> 1).astype(np.int16)
    dadj = dstloc[order2] + 128.0 * (s2 & 1)
    meta2_np, idx2_np = _fill_meta_idx(
        core2, slot2, dadj, norm_all[order2], srcp, TT2, S2
    )

    layout = (tiles1, tiles2)
    return layout, meta1_np, idx1_np, meta2_np, idx2_np


def _build(layout, ablate="full"):
    """Build the (single, SPMD) Bacc program for the given tile counts.

    ablate: "full" | "nocc" (local copy instead of AllGather) |
    "nogather" (memset instead of dma_gather) | "l1only" | "l2only".
    Non-"full" variants produce wrong results; timing probes only.
    """
    do_l1 = ablate != "l2only"
    do_l2 = ablate != "l1only"
    do_cc = ablate not in ("nocc", "l1only")
    do_gather = ablate != "nogather"
    tiles1, tiles2 = layout
    TT1, tbase1, seg1 = _layout1(tiles1)
    S1 = TT1 * BLK
    tbase2 = np.concatenate([[0], np.cumsum(tiles2)[:-1]]).astype(np.int64)
    TT2 = int(tiles2.sum())
    S2 = TT2 * BLK
    f32 = mybir.dt.float32
    bf16 = mybir.dt.bfloat16
    i16 = mybir.dt.int16
    AF = mybir.ActivationFunctionType
    OP = mybir.AluOpType

    nc = bacc.Bacc(
        "TRN2",
        target_bir_lowering=False,
        debug=False,
        num_devices=NCORES,
        num_swdge_queues=4,
    )
    xin = nc.dram_tensor("xin", [N, FIN], bf16, kind="ExternalInput")
    w1 = nc.dram_tensor("w1", [FIN, HID], bf16, kind="ExternalInput")
    w2 = nc.dram_tensor("w2", [HID, FOUT], bf16, kind="ExternalInput")
    b1c = nc.dram_tensor("b1c", [HID, 1], f32, kind="ExternalInput")
    b2r = nc.dram_tensor("b2r", [128, FOUT], f32, kind="ExternalInput")
    iot = nc.dram_tensor("iot", [128, BLK], bf16, kind="ExternalInput")
    io2 = nc.dram_tensor("io2", [128, 2 * BLK], bf16, kind="ExternalInput")
    idn = nc.dram_tensor("idn", [128, 128], bf16, kind="ExternalInput")
    meta1 = nc.dram_tensor("meta1", [128, TT1, 2], f32, kind="ExternalInput")
    idx1t = nc.dram_tensor("idx1", [128, S1 // 16], i16, kind="ExternalInput")
    meta2 = nc.dram_tensor("meta2", [128, TT2, 2], f32, kind="ExternalInput")
    idx2t = nc.dram_tensor("idx2", [128, S2 // 16], i16, kind="ExternalInput")
    outp = nc.dram_tensor("out", [NSH, FOUT], f32, kind="ExternalOutput")

    with tile.TileContext(nc) as tc, ExitStack() as ctx:
        const = ctx.enter_context(tc.tile_pool(name="const", bufs=1))
        dram = ctx.enter_context(tc.tile_pool(name="dram", bufs=1, space="DRAM"))
        gpool = ctx.enter_context(tc.tile_pool(name="g", bufs=3))
        g2pool = ctx.enter_context(tc.tile_pool(name="g2", bufs=3))
        tpp = ctx.enter_context(tc.tile_pool(name="tp", bufs=12))
        evp = ctx.enter_context(tc.tile_pool(name="ev", bufs=4))
        psa = ctx.enter_context(tc.tile_pool(name="psa", bufs=3, space="PSUM"))
        psd = ctx.enter_context(tc.tile_pool(name="psd", bufs=1, space="PSUM"))
        pst = ctx.enter_context(tc.tile_pool(name="pst", bufs=1, space="PSUM"))
        pso = ctx.enter_context(tc.tile_pool(name="pso", bufs=1, space="PSUM"))
        psb = ctx.enter_context(tc.tile_pool(name="psb", bufs=2, space="PSUM"))

        def cload(ap, shape, dtype, tag):
            t = const.tile(shape, dtype, tag=tag)
            nc.sync.dma_start(t[:], ap)
            return t

        w1_sb = cload(w1[:, :], [FIN, HID], bf16, "w1")
        w2_sb = cload(w2[:, :], [HID, FOUT], bf16, "w2")
        b1_sb = cload(b1c[:, :], [HID, 1], f32, "b1")
        b2_sb = cload(b2r[:, :], [128, FOUT], f32, "b2")
        iota_sb = cload(iot[:, :], [128, BLK], bf16, "iota")
        io2_sb = cload(io2[:, :], [128, 2 * BLK], bf16, "io2")
        idn_sb = cload(idn[:, :], [128, 128], bf16, "idn")
        meta1_sb = cload(meta1[:, :, :], [128, TT1, 2], f32, "meta1")
        idx1_sb = cload(idx1t[:, :], [128, S1 // 16], i16, "idx1")
        meta2_sb = cload(meta2[:, :, :], [128, TT2, 2], f32, "meta2")
        idx2_sb = cload(idx2t[:, :], [128, S2 // 16], i16, "idx2")

        t2loc = dram.tile([NSH, FOUT], bf16, tag="t2loc")
        t2full = dram.tile([N, FOUT], bf16, tag="t2full", addr_space="Shared")

        qctr = [0]  # round-robin gather queue assignment

        def next_q():
            q = qctr[0] % 4
            qctr[0] += 1
            return q

        def finalize1(b, ps):
            """ps [x=128, d=128] f32 -> h2 = relu((Ahat X W1)+b1) @ W2 -> t2loc."""
            ag = evp.tile([128, BLK], bf16, tag="ag")
            nc.scalar.activation(ag[:], ps[:], AF.Copy)
            hp = psd.tile([128, HID], f32, tag="hp")
            nc.tensor.matmul(hp[:], lhsT=ag[:], rhs=w1_sb[:], start=True, stop=True)
            hpe = evp.tile([128, HID], bf16, tag="hpe")
            nc.scalar.activation(hpe[:], hp[:], AF.Copy)
            pT = pst.tile([128, BLK], bf16, tag="pT")
            nc.tensor.transpose(pT[:], hpe[:], idn_sb[:])
            ev = evp.tile([128, BLK], bf16, tag="evt")
            nc.scalar.activation(ev[:], pT[:], AF.Relu, bias=b1_sb[:, 0:1])
            h2p = pso.tile([128, FOUT], f32, tag="h2p")
            nc.tensor.matmul(h2p[:], lhsT=ev[:], rhs=w2_sb[:], start=True, stop=True)
            h2 = evp.tile([128, FOUT], bf16, tag="h2")
            nc.scalar.activation(h2[:], h2p[:], AF.Copy)
            r0 = b * BLK
            r1 = min(NSH, r0 + BLK)
            nc.sync.dma_start(t2loc[r0:r1, :], h2[0 : r1 - r0, :])

        # ---- layer 1: aggregate raw X rows, then per-block W1/relu/W2 ----
        for sb in range(NSB if do_l1 else 0):
            blocks = list(range(sb * SBW, min((sb + 1) * SBW, NBLK)))
            gt = {}
            for h in (0, 1):
                t0, ntl = seg1[(sb, h)]
                if ntl == 0:
                    continue
                g = gpool.tile([128, ntl, FIN], bf16, tag=f"g{h}")
                view = xin[0:HALF, :] if h == 0 else xin[HALF : 2 * HALF, :]
                if not do_gather:
                    nc.vector.memset(g[:], 0)
                for q0 in range(0, ntl, GCH) if do_gather else ():
                    qn = min(GCH, ntl - q0)
                    c0 = (t0 + q0) * 8  # idx columns (tile*128/16)
                    nc.gpsimd.dma_gather(
                        out_ap=g[:, q0 : q0 + qn, :],
                        in_ap=view,
                        idxs_ap=idx1_sb[:, c0 : c0 + qn * 8],
                        num_idxs=qn * 128,
                        num_idxs_reg=qn * 128,
                        elem_size=FIN,
                        queue_num=next_q(),
                    )
                gt[h] = (g, t0)
            for b in blocks:
                ps = psa.tile([128, BLK], f32, tag="psa")
                for h in (0, 1):
                    if seg1[(sb, h)][1] == 0 or tiles1[b][h] == 0:
                        continue
                    g, seg_t0 = gt[h]
                    for k in range(int(tiles1[b][h])):
                        t = int(tbase1[b][h]) + k
                        first = (k == 0) and (h == 0 or tiles1[b][0] == 0)
                        last = (k == int(tiles1[b][h]) - 1) and (
                            h == 1 or tiles1[b][1] == 0
                        )
                        tp = tpp.tile([128, BLK], bf16, tag="tp")
                        nc.vector.tensor_scalar(
                            out=tp[:],
                            in0=iota_sb[:],
                            scalar1=meta1_sb[:, t, 0:1],
                            scalar2=meta1_sb[:, t, 1:2],
                            op0=OP.is_equal,
                            op1=OP.mult,
                        )
                        nc.tensor.matmul(
                            ps[:],
                            lhsT=g[:, t - seg_t0, :],
                            rhs=tp[:],
                            start=first,
                            stop=last,
                        )
                finalize1(b, ps)

        # ---- exchange the pair-packed h2 table ----
        if do_cc:
            nc.gpsimd.collective_compute(
                "AllGather",
                mybir.AluOpType.bypass,
                replica_groups=[list(range(NCORES))],
                ins=[t2loc[:].opt()],
                outs=[t2full[:].opt()],
            )
        elif do_l1 and do_l2:
            nc.sync.dma_start(t2full[0:NSH, :], t2loc[:, :])
        t2pair = t2full[:, :].rearrange("(a b) c -> a (b c)", b=2)  # [25000, 128]

        # ---- layer 2: gather node pairs, 256-wide one-hot, two matmuls ----
        block_of_tile2 = np.repeat(np.arange(NBLK), tiles2)
        g2 = None
        ps2 = None
        for t in range(TT2 if do_l2 else 0):
            if t % GCH == 0:
                qn = min(GCH, TT2 - t)
                g2 = g2pool.tile([128, GCH, FIN], bf16, tag="g2")
                if do_gather:
                    nc.gpsimd.dma_gather(
                        out_ap=g2[:, 0:qn, :],
                        in_ap=t2pair,
                        idxs_ap=idx2_sb[:, t * 8 : (t + qn) * 8],
                        num_idxs=qn * 128,
                        num_idxs_reg=qn * 128,
                        elem_size=FIN,
                        queue_num=next_q(),
                    )
                else:
                    nc.vector.memset(g2[:], 0)
            b = int(block_of_tile2[t])
            k = t - int(tbase2[b])
            first = k == 0
            last = k == int(tiles2[b]) - 1
            if first:
                ps2 = psb.tile([128, FOUT], f32, tag="psb")
            tp2 = tpp.tile([128, 2 * BLK], bf16, tag="tp2")
            nc.vector.tensor_scalar(
                out=tp2[:],
                in0=io2_sb[:],
                scalar1=meta2_sb[:, t, 0:1],
                scalar2=meta2_sb[:, t, 1:2],
                op0=OP.is_equal,
                op1=OP.mult,
            )
            j = t % GCH
            nc.tensor.matmul(
                ps2[:],
                lhsT=tp2[:, 0:BLK],
                rhs=g2[:, j, 0:FOUT],
                start=first,
                stop=False,
            )
            nc.tensor.matmul(
                ps2[:],
                lhsT=tp2[:, BLK : 2 * BLK],
                rhs=g2[:, j, FOUT:FIN],
                start=False,
                stop=last,
            )
            if last:
                oo = evp.tile([128, FOUT], f32, tag="oo")
                nc.vector.tensor_add(oo[:], ps2[:], b2_sb[:])
                r0 = b * BLK
                r1 = min(NSH, r0 + BLK)
                nc.sync.dma_start(outp[r0:r1, :], oo[0 : r1 - r0, :])

    nc.finalize()
    return nc


def _in_maps(x, W1, b1, W2, b2, prep):
    layout, meta1_np, idx1_np, meta2_np, idx2_np = prep
    xb = np.asarray(x, np.float32).astype(BF16)
    w1b = np.asarray(W1, np.float32).astype(BF16)
    w2b = np.asarray(W2, np.float32).astype(BF16)
    b1col = np.asarray(b1, np.float32).reshape(HID, 1).copy()
    b2rep = np.broadcast_to(np.asarray(b2, np.float32), (128, FOUT)).copy()
    iota = np.broadcast_to(np.arange(BLK, dtype=np.float32), (128, BLK)).astype(BF16)
    io2 = np.broadcast_to(np.arange(2 * BLK, dtype=np.float32), (128, 2 * BLK)).astype(
        BF16
    )
    idn = np.eye(128, dtype=np.float32).astype(BF16)
    shared = {
        "xin": np.ascontiguousarray(xb),
        "w1": w1b,
        "w2": w2b,
        "b1c": b1col,
        "b2r": b2rep,
        "iot": np.ascontiguousarray(iota),
        "io2": np.ascontiguousarray(io2),
        "idn": idn,
    }
    return [
        dict(
            shared,
            meta1=np.ascontiguousarray(meta1_np[c]),
            idx1=np.ascontiguousarray(idx1_np[c]),
            meta2=np.ascontiguousarray(meta2_np[c]),
            idx2=np.ascontiguousarray(idx2_np[c]),
        )
        for c in range(NCORES)
    ]


def kernel(x, edge_index, W1, b1, W2, b2):
    prep = _prep(edge_index)
    nc = _build(prep[0])
    in_maps = _in_maps(x, W1, b1, W2, b2, prep)
    res = run_bass_kernel_spmd(nc, in_maps, core_ids=list(range(NCORES)), trace=False)
    out = np.concatenate(
        [res.results[c]["out"].astype(np.float32) for c in range(NCORES)], axis=0
    )
    return out


# revision 22
# speedup vs baseline: 2.0500x; 1.4109x over previous
"""Two-layer GCN (PyG GCNConv semantics) on 8 Trainium2 NeuronCores.

Math: out = Ahat @ relu(Ahat @ (X@W1) + b1) @ W2 + b2, with
Ahat = D^-1/2 (A + I) D^-1/2.

Restructured vs the phase-A baseline:
  - Layer 1 aggregates RAW X rows (Ahat@X) gathered straight from the
    input table, then applies W1 per 128-row destination block on-chip
    ((Ahat X) W1 == Ahat (X W1)); no replicated dense pre-pass, no
    table1 spill/reload.
  - The block result is relu'd (bias folded into the activation after a
    PE transpose) and immediately multiplied by W2, so the collective
    exchanges the 64-wide h2 = relu(.)@W2 table (6.4 MB instead of
    12.8 MB).  Aggregation is linear, so Ahat(h)W2 == Ahat(h W2).
  - The h2 table is PAIR-PACKED [25000, 128] bf16 (dma_gather needs
    256B-multiple rows): layer-2 gathers fetch a node pair, and a
    256-wide one-hot (dst + 128*parity) feeds two matmuls that pick the
    correct half.  Pair indices fit int16, so layer 2 needs no halves.

Sharding: destination nodes split across 8 cores (6250 each); one
AllGather (Shared output) shares the layer-2 source table.
"""

import sys

import numpy as np

try:
    import concourse.bass as bass  # noqa: F401
except ImportError:
    sys.path.insert(0, "/opt/trn_rl_repo")

from contextlib import ExitStack

import ml_dtypes

import concourse.bass as bass
import concourse.tile as tile
from concourse import bacc, mybir
from concourse.bass_utils import run_bass_kernel_spmd

BF16 = ml_dtypes.bfloat16

N = 50000
E = 800000
FIN = 128
HID = 128
FOUT = 64
NCORES = 8
NSH = N // NCORES  # 6250 destination nodes per core
BLK = 128  # dst block (psum window)
NBLK = (NSH + BLK - 1) // BLK  # 49
SBW = 4  # dst blocks per superblock (layer-1 gather segmenting)
NSB = (NBLK + SBW - 1) // SBW  # 13
HALF = 25000  # layer-1 table half split (int16 gather indices)
GCH = 8  # tiles per dma_gather chunk (8*128 = 1024 = SWDGE ring)
SPLITB = 24  # first layer-2 chunk = dst blocks [0, SPLITB) of every core
RA = SPLITB * BLK  # 3072 rows per core in chunk A
RB = NSH - RA  # 3178 rows per core in chunk B
CCSB = 5  # fire AllGather-A after this superblock (covers blocks 0..23)


def _layout1(tiles):
    """Layer-1 static layout from per-(block,half) tile counts.

    Returns (TT, tile_base[NBLK][2], seg: {(sb,h): (tile0, ntiles)}).
    Data/program order: for sb, for half, for block in sb, k tiles.
    """
    tile_base = np.zeros((NBLK, 2), dtype=np.int64)
    seg = {}
    pos = 0
    for sb in range(NSB):
        blocks = range(sb * SBW, min((sb + 1) * SBW, NBLK))
        for h in (0, 1):
            seg_start = pos
            for b in blocks:
                tile_base[b][h] = pos
                pos += int(tiles[b][h])
            seg[(sb, h)] = (seg_start, pos - seg_start)
    return int(pos), tile_base, seg


def _rank_within_groups(gid):
    change = np.r_[True, gid[1:] != gid[:-1]]
    gstart = np.maximum.accumulate(np.where(change, np.arange(len(gid)), 0))
    return np.arange(len(gid)) - gstart


def _fill_meta_idx(core_s, slot, dval, nval, srcv, TT, S):
    """Build per-core meta [128, TT, 2] f32 and idx [128, S//16] i16 tables."""
    meta_np = np.zeros((NCORES, 128, TT, 2), dtype=np.float32)
    idx_np = np.zeros((NCORES, 128, S // 16), dtype=np.int16)
    for c in range(NCORES):
        m = core_s == c
        sl = slot[m]
        tt = sl // BLK
        pp = sl % BLK
        meta_np[c, pp, tt, 0] = dval[m]
        meta_np[c, pp, tt, 1] = nval[m]
        col = sl // 16
        row = sl % 16
        v = srcv[m]
        for g in range(8):  # replicate across the 8 gpsimd 16-partition groups
            idx_np[c, row + 16 * g, col] = v
    return meta_np, idx_np


def _prep(edge_index):
    src = np.asarray(edge_index[0], dtype=np.int64)
    dst = np.asarray(edge_index[1], dtype=np.int64)
    deg = (np.bincount(dst, minlength=N) + 1).astype(np.float64)
    dinv = (1.0 / np.sqrt(deg)).astype(np.float32)

    s_all = np.concatenate([src, np.arange(N, dtype=np.int64)])
    d_all = np.concatenate([dst, np.arange(N, dtype=np.int64)])
    norm_all = dinv[s_all] * dinv[d_all]
    core = d_all // NSH
    local = d_all % NSH
    block = local // BLK
    dstloc = (local % BLK).astype(np.float32)

    # ---- layer 1: halves (int16 src index into x halves), sb segments ----
    half = (s_all >= HALF).astype(np.int64)
    sbk = block // SBW
    cidx = (core * NBLK + block) * 2 + half
    cnt = np.bincount(cidx, minlength=NCORES * NBLK * 2).reshape(NCORES, NBLK, 2)
    tiles1 = ((cnt + BLK - 1) // BLK).max(axis=0)  # [NBLK, 2]
    TT1, tbase1, seg1 = _layout1(tiles1)
    S1 = TT1 * BLK

    order = np.lexsort((local, block, half, sbk, core))
    s_s = s_all[order]
    core_s = core[order]
    block_s = block[order]
    half_s = half[order]
    gid = (core_s * NBLK + block_s) * 2 + half_s
    rank = _rank_within_groups(gid)
    slot1 = tbase1[block_s, half_s] * BLK + rank
    src_loc = np.where(half_s == 0, s_s, s_s - HALF).astype(np.int16)
    meta1_np, idx1_np = _fill_meta_idx(
        core_s, slot1, dstloc[order], norm_all[order], src_loc, TT1, S1
    )

    # ---- layer 2: pair-packed split tables (A: src rows [0,RA) of each
    # core, B: the rest), phase-major so table-A work overlaps AllGather-B
    s_core = s_all // NSH
    s_r = s_all % NSH
    h2 = (s_r >= RA).astype(np.int64)
    cidx2 = (core * NBLK + block) * 2 + h2
    cnt2 = np.bincount(cidx2, minlength=NCORES * NBLK * 2).reshape(NCORES, NBLK, 2)
    tiles2 = ((cnt2 + BLK - 1) // BLK).max(axis=0)  # [NBLK, 2]
    TT2a = int(tiles2[:, 0].sum())
    TT2 = TT2a + int(tiles2[:, 1].sum())
    S2 = TT2 * BLK
    tbase2 = np.zeros((NBLK, 2), dtype=np.int64)
    tbase2[:, 0] = np.concatenate([[0], np.cumsum(tiles2[:, 0])[:-1]])
    tbase2[:, 1] = TT2a + np.concatenate([[0], np.cumsum(tiles2[:, 1])[:-1]])

    order2 = np.lexsort((local, block, h2, core))
    s2c = s_core[order2]
    s2r = s_r[order2]
    h2s = h2[order2]
    core2 = core[order2]
    block2 = block[order2]
    gid2 = (core2 * NBLK + block2) * 2 + h2s
    rank2 = _rank_within_groups(gid2)
    slot2 = tbase2[block2, h2s] * BLK + rank2
    srcp = np.where(
        h2s == 0, s2c * (RA // 2) + (s2r >> 1), s2c * (RB // 2) + ((s2r - RA) >> 1)
    ).astype(np.int16)
    dadj = dstloc[order2] + 128.0 * (s2r & 1)
    meta2_np, idx2_np = _fill_meta_idx(
        core2, slot2, dadj, norm_all[order2], srcp, TT2, S2
    )

    layout = (tiles1, tiles2)
    return layout, meta1_np, idx1_np, meta2_np, idx2_np


def _build(layout, ablate="full"):
    """Build the (single, SPMD) Bacc program for the given tile counts.

    ablate: "full" | "nocc" (local copy instead of AllGather) |
    "nogather" (memset instead of dma_gather) | "l1only" | "l2only".
    Non-"full" variants produce wrong results; timing probes only.
    """
    do_l1 = ablate != "l2only"
    do_l2 = ablate != "l1only"
    do_cc = ablate not in ("nocc", "l1only")
    do_gather = ablate != "nogather"
    tiles1, tiles2 = layout
    TT1, tbase1, seg1 = _layout1(tiles1)
    S1 = TT1 * BLK
    TT2a = int(tiles2[:, 0].sum())
    TT2 = TT2a + int(tiles2[:, 1].sum())
    S2 = TT2 * BLK
    tbase2 = np.zeros((NBLK, 2), dtype=np.int64)
    tbase2[:, 0] = np.concatenate([[0], np.cumsum(tiles2[:, 0])[:-1]])
    tbase2[:, 1] = TT2a + np.concatenate([[0], np.cumsum(tiles2[:, 1])[:-1]])
    f32 = mybir.dt.float32
    bf16 = mybir.dt.bfloat16
    i16 = mybir.dt.int16
    AF = mybir.ActivationFunctionType
    OP = mybir.AluOpType

    nc = bacc.Bacc(
        "TRN2",
        target_bir_lowering=False,
        debug=False,
        num_devices=NCORES,
        num_swdge_queues=4,
    )
    xin = nc.dram_tensor("xin", [N, FIN], bf16, kind="ExternalInput")
    w1 = nc.dram_tensor("w1", [FIN, HID], bf16, kind="ExternalInput")
    w2 = nc.dram_tensor("w2", [HID, FOUT], bf16, kind="ExternalInput")
    b1c = nc.dram_tensor("b1c", [HID, 1], f32, kind="ExternalInput")
    b2r = nc.dram_tensor("b2r", [128, FOUT], f32, kind="ExternalInput")
    iot = nc.dram_tensor("iot", [128, BLK], bf16, kind="ExternalInput")
    io2 = nc.dram_tensor("io2", [128, 2 * BLK], bf16, kind="ExternalInput")
    idn = nc.dram_tensor("idn", [128, 128], bf16, kind="ExternalInput")
    meta1 = nc.dram_tensor("meta1", [128, TT1, 2], f32, kind="ExternalInput")
    idx1t = nc.dram_tensor("idx1", [128, S1 // 16], i16, kind="ExternalInput")
    meta2 = nc.dram_tensor("meta2", [128, TT2, 2], f32, kind="ExternalInput")
    idx2t = nc.dram_tensor("idx2", [128, S2 // 16], i16, kind="ExternalInput")
    outp = nc.dram_tensor("out", [NSH, FOUT], f32, kind="ExternalOutput")

    with tile.TileContext(nc) as tc, ExitStack() as ctx:
        const = ctx.enter_context(tc.tile_pool(name="const", bufs=1))
        dram = ctx.enter_context(tc.tile_pool(name="dram", bufs=1, space="DRAM"))
        gpool = ctx.enter_context(tc.tile_pool(name="g", bufs=3))
        g2pool = ctx.enter_context(tc.tile_pool(name="g2", bufs=3))
        tpp = ctx.enter_context(tc.tile_pool(name="tp", bufs=12))
        evp = ctx.enter_context(tc.tile_pool(name="ev", bufs=4))
        accp = ctx.enter_context(tc.tile_pool(name="acc", bufs=1))
        psa = ctx.enter_context(tc.tile_pool(name="psa", bufs=3, space="PSUM"))
        psd = ctx.enter_context(tc.tile_pool(name="psd", bufs=1, space="PSUM"))
        pst = ctx.enter_context(tc.tile_pool(name="pst", bufs=1, space="PSUM"))
        pso = ctx.enter_context(tc.tile_pool(name="pso", bufs=1, space="PSUM"))
        psb = ctx.enter_context(tc.tile_pool(name="psb", bufs=2, space="PSUM"))

        def cload(ap, shape, dtype, tag):
            t = const.tile(shape, dtype, tag=tag)
            nc.sync.dma_start(t[:], ap)
            return t

        w1_sb = cload(w1[:, :], [FIN, HID], bf16, "w1")
        w2_sb = cload(w2[:, :], [HID, FOUT], bf16, "w2")
        b1_sb = cload(b1c[:, :], [HID, 1], f32, "b1")
        b2_sb = cload(b2r[:, :], [128, FOUT], f32, "b2")
        iota_sb = cload(iot[:, :], [128, BLK], bf16, "iota")
        io2_sb = cload(io2[:, :], [128, 2 * BLK], bf16, "io2")
        idn_sb = cload(idn[:, :], [128, 128], bf16, "idn")
        meta1_sb = cload(meta1[:, :, :], [128, TT1, 2], f32, "meta1")
        idx1_sb = cload(idx1t[:, :], [128, S1 // 16], i16, "idx1")
        meta2_sb = cload(meta2[:, :, :], [128, TT2, 2], f32, "meta2")
        idx2_sb = cload(idx2t[:, :], [128, S2 // 16], i16, "idx2")

        t2locA = dram.tile([RA, FOUT], bf16, tag="t2locA")
        t2locB = dram.tile([RB, FOUT], bf16, tag="t2locB")
        t2fA = dram.tile([NCORES * RA, FOUT], bf16, tag="t2fA", addr_space="Shared")
        t2fB = dram.tile([NCORES * RB, FOUT], bf16, tag="t2fB", addr_space="Shared")

        qctr = [0]  # round-robin gather queue assignment

        def next_q():
            q = qctr[0] % 4
            qctr[0] += 1
            return q

        def finalize1(b, ps):
            """ps [x=128, d=128] f32 -> h2 = relu((Ahat X W1)+b1) @ W2 -> t2loc."""
            ag = evp.tile([128, BLK], bf16, tag="ag")
            nc.scalar.activation(ag[:], ps[:], AF.Copy)
            hp = psd.tile([128, HID], f32, tag="hp")
            nc.tensor.matmul(hp[:], lhsT=ag[:], rhs=w1_sb[:], start=True, stop=True)
            hpe = evp.tile([128, HID], bf16, tag="hpe")
            nc.scalar.activation(hpe[:], hp[:], AF.Copy)
            pT = pst.tile([128, BLK], bf16, tag="pT")
            nc.tensor.transpose(pT[:], hpe[:], idn_sb[:])
            ev = evp.tile([128, BLK], bf16, tag="evt")
            nc.scalar.activation(ev[:], pT[:], AF.Relu, bias=b1_sb[:, 0:1])
            h2p = pso.tile([128, FOUT], f32, tag="h2p")
            nc.tensor.matmul(h2p[:], lhsT=ev[:], rhs=w2_sb[:], start=True, stop=True)
            h2 = evp.tile([128, FOUT], bf16, tag="h2")
            nc.scalar.activation(h2[:], h2p[:], AF.Copy)
            r0 = b * BLK
            r1 = min(NSH, r0 + BLK)
            if b < SPLITB:
                nc.sync.dma_start(t2locA[r0:r1, :], h2[0 : r1 - r0, :])
            else:
                nc.sync.dma_start(t2locB[r0 - RA : r1 - RA, :], h2[0 : r1 - r0, :])

        def ccgather(ins_ap, outs_ap):
            nc.gpsimd.collective_compute(
                "AllGather",
                mybir.AluOpType.bypass,
                replica_groups=[list(range(NCORES))],
                ins=[ins_ap],
                outs=[outs_ap],
            )

        # ---- layer 1: aggregate raw X rows, then per-block W1/relu/W2 ----
        for sb in range(NSB if do_l1 else 0):
            if sb == CCSB + 1 and do_cc:
                # chunk-A table exchange; overlaps the layer-1 tail
                ccgather(t2locA[:].opt(), t2fA[:].opt())
            blocks = list(range(sb * SBW, min((sb + 1) * SBW, NBLK)))
            gt = {}
            for h in (0, 1):
                t0, ntl = seg1[(sb, h)]
                if ntl == 0:
                    continue
                g = gpool.tile([128, ntl, FIN], bf16, tag=f"g{h}")
                view = xin[0:HALF, :] if h == 0 else xin[HALF : 2 * HALF, :]
                if not do_gather:
                    nc.vector.memset(g[:], 0)
                for q0 in range(0, ntl, GCH) if do_gather else ():
                    qn = min(GCH, ntl - q0)
                    c0 = (t0 + q0) * 8  # idx columns (tile*128/16)
                    nc.gpsimd.dma_gather(
                        out_ap=g[:, q0 : q0 + qn, :],
                        in_ap=view,
                        idxs_ap=idx1_sb[:, c0 : c0 + qn * 8],
                        num_idxs=qn * 128,
                        num_idxs_reg=qn * 128,
                        elem_size=FIN,
                        queue_num=next_q(),
                    )
                gt[h] = (g, t0)
            for b in blocks:
                ps = psa.tile([128, BLK], f32, tag="psa")
                for h in (0, 1):
                    if seg1[(sb, h)][1] == 0 or tiles1[b][h] == 0:
                        continue
                    g, seg_t0 = gt[h]
                    for k in range(int(tiles1[b][h])):
                        t = int(tbase1[b][h]) + k
                        first = (k == 0) and (h == 0 or tiles1[b][0] == 0)
                        last = (k == int(tiles1[b][h]) - 1) and (
                            h == 1 or tiles1[b][1] == 0
                        )
                        tp = tpp.tile([128, BLK], bf16, tag="tp")
                        nc.vector.tensor_scalar(
                            out=tp[:],
                            in0=iota_sb[:],
                            scalar1=meta1_sb[:, t, 0:1],
                            scalar2=meta1_sb[:, t, 1:2],
                            op0=OP.is_equal,
                            op1=OP.mult,
                        )
                        nc.tensor.matmul(
                            ps[:],
                            lhsT=g[:, t - seg_t0, :],
                            rhs=tp[:],
                            start=first,
                            stop=last,
                        )
                finalize1(b, ps)

        # pair views of the exchanged chunk tables: [rows/2, 128]
        t2pA = t2fA[:, :].rearrange("(a b) c -> a (b c)", b=2)
        t2pB = t2fB[:, :].rearrange("(a b) c -> a (b c)", b=2)
        if not do_cc and do_l1 and do_l2:
            nc.sync.dma_start(t2fA[0:RA, :], t2locA[:, :])
            nc.sync.dma_start(t2fB[0:RB, :], t2locB[:, :])

        # ---- layer 2: gather node pairs, 256-wide one-hot, two matmuls.
        # Phase 0 consumes table A (runs while AllGather-B is in flight,
        # accumulating per-block partials in SBUF); phase 1 consumes B.
        def writeout(b, oo):
            r0 = b * BLK
            r1 = min(NSH, r0 + BLK)
            nc.sync.dma_start(outp[r0:r1, :], oo[0 : r1 - r0, :])

        acc = {}
        for h in (0, 1) if do_l2 else ():
            if h == 1 and do_cc:
                ccgather(t2locB[:].opt(), t2fB[:].opt())
            view = t2pA if h == 0 else t2pB
            pt0 = 0 if h == 0 else TT2a
            pt1 = TT2a if h == 0 else TT2
            btile = np.repeat(np.arange(NBLK), tiles2[:, h])
            g2 = None
            ps2 = None
            for t in range(pt0, pt1):
                if (t - pt0) % GCH == 0:
                    qn = min(GCH, pt1 - t)
                    g2 = g2pool.tile([128, GCH, FIN], bf16, tag="g2")
                    if do_gather:
                        nc.gpsimd.dma_gather(
                            out_ap=g2[:, 0:qn, :],
                            in_ap=view,
                            idxs_ap=idx2_sb[:, t * 8 : (t + qn) * 8],
                            num_idxs=qn * 128,
                            num_idxs_reg=qn * 128,
                            elem_size=FIN,
                            queue_num=next_q(),
                        )
                    else:
                        nc.vector.memset(g2[:], 0)
                b = int(btile[t - pt0])
                k = t - int(tbase2[b][h])
                first = k == 0
                last = k == int(tiles2[b][h]) - 1
                if first:
                    ps2 = psb.tile([128, FOUT], f32, tag="psb")
                tp2 = tpp.tile([128, 2 * BLK], bf16, tag="tp2")
                nc.vector.tensor_scalar(
                    out=tp2[:],
                    in0=io2_sb[:],
                    scalar1=meta2_sb[:, t, 0:1],
                    scalar2=meta2_sb[:, t, 1:2],
                    op0=OP.is_equal,
                    op1=OP.mult,
                )
                j = (t - pt0) % GCH
                nc.tensor.matmul(
                    ps2[:],
                    lhsT=tp2[:, 0:BLK],
                    rhs=g2[:, j, 0:FOUT],
                    start=first,
                    stop=False,
                )
                nc.tensor.matmul(
                    ps2[:],
                    lhsT=tp2[:, BLK : 2 * BLK],
                    rhs=g2[:, j, FOUT:FIN],
                    start=False,
                    stop=last,
                )
                if not last:
                    continue
                if h == 0:
                    if tiles2[b][1] > 0:
                        ab = accp.tile([128, FOUT], f32, tag=f"acc{b}")
                        nc.vector.tensor_add(ab[:], ps2[:], b2_sb[:])
                        acc[b] = ab
                    else:
                        oo = evp.tile([128, FOUT], f32, tag="oo")
                        nc.vector.tensor_add(oo[:], ps2[:], b2_sb[:])
                        writeout(b, oo)
                else:
                    oo = evp.tile([128, FOUT], f32, tag="oo")
                    nc.vector.tensor_add(
                        oo[:], ps2[:], acc[b][:] if b in acc else b2_sb[:]
                    )
                    writeout(b, oo)

    nc.finalize()
    return nc


def _in_maps(x, W1, b1, W2, b2, prep):
    layout, meta1_np, idx1_np, meta2_np, idx2_np = prep
    xb = np.asarray(x, np.float32).astype(BF16)
    w1b = np.asarray(W1, np.float32).astype(BF16)
    w2b = np.asarray(W2, np.float32).astype(BF16)
    b1col = np.asarray(b1, np.float32).reshape(HID, 1).copy()
    b2rep = np.broadcast_to(np.asarray(b2, np.float32), (128, FOUT)).copy()
    iota = np.broadcast_to(np.arange(BLK, dtype=np.float32), (128, BLK)).astype(BF16)
    io2 = np.broadcast_to(np.arange(2 * BLK, dtype=np.float32), (128, 2 * BLK)).astype(
        BF16
    )
    idn = np.eye(128, dtype=np.float32).astype(BF16)
    shared = {
        "xin": np.ascontiguousarray(xb),
        "w1": w1b,
        "w2": w2b,
        "b1c": b1col,
        "b2r": b2rep,
        "iot": np.ascontiguousarray(iota),
        "io2": np.ascontiguousarray(io2),
        "idn": idn,
    }
    return [
        dict(
            shared,
            meta1=np.ascontiguousarray(meta1_np[c]),
            idx1=np.ascontiguousarray(idx1_np[c]),
            meta2=np.ascontiguousarray(meta2_np[c]),
            idx2=np.ascontiguousarray(idx2_np[c]),
        )
        for c in range(NCORES)
    ]


def kernel(x, edge_index, W1, b1, W2, b2):
    prep = _prep(edge_index)
    nc = _build(prep[0])
    in_maps = _in_maps(x, W1, b1, W2, b2, prep)
    res = run_bass_kernel_spmd(nc, in_maps, core_ids=list(range(NCORES)), trace=False)
    out = np.concatenate(
        [res.results[c]["out"].astype(np.float32) for c in range(NCORES)], axis=0
    )
    return out


# revision 26
# speedup vs baseline: 2.3971x; 1.1693x over previous
"""Two-layer GCN (PyG GCNConv semantics) on 8 Trainium2 NeuronCores.

Math: out = Ahat @ relu(Ahat @ (X@W1) + b1) @ W2 + b2, with
Ahat = D^-1/2 (A + I) D^-1/2.

Restructured vs the phase-A baseline:
  - Layer 1 aggregates RAW X rows (Ahat@X) gathered straight from the
    input table, then applies W1 per 128-row destination block on-chip
    ((Ahat X) W1 == Ahat (X W1)); no replicated dense pre-pass, no
    table1 spill/reload.
  - The block result is relu'd (bias folded into the activation after a
    PE transpose) and immediately multiplied by W2, so the collective
    exchanges the 64-wide h2 = relu(.)@W2 table (6.4 MB instead of
    12.8 MB).  Aggregation is linear, so Ahat(h)W2 == Ahat(h W2).
  - The h2 table is PAIR-PACKED [25000, 128] bf16 (dma_gather needs
    256B-multiple rows): layer-2 gathers fetch a node pair, and a
    256-wide one-hot (dst + 128*parity) feeds two matmuls that pick the
    correct half.  Pair indices fit int16, so layer 2 needs no halves.

Sharding: destination nodes split across 8 cores (6250 each); one
AllGather (Shared output) shares the layer-2 source table.
"""

import sys

import numpy as np

try:
    import concourse.bass as bass  # noqa: F401
except ImportError:
    sys.path.insert(0, "/opt/trn_rl_repo")

from contextlib import ExitStack

import ml_dtypes

import concourse.bass as bass
import concourse.tile as tile
from concourse import bacc, mybir
from concourse.bass_utils import run_bass_kernel_spmd

BF16 = ml_dtypes.bfloat16

N = 50000
E = 800000
FIN = 128
HID = 128
FOUT = 64
NCORES = 8
NSH = N // NCORES  # 6250 destination nodes per core
BLK = 128  # dst block (psum window)
NBLK = (NSH + BLK - 1) // BLK  # 49
SBW = 4  # dst blocks per superblock (layer-1 gather segmenting)
NSB = (NBLK + SBW - 1) // SBW  # 13
HALF = 25000  # layer-1 table half split (int16 gather indices)
GCH = 8  # tiles per dma_gather chunk (8*128 = 1024 = SWDGE ring)
SPLITB = 24  # first layer-2 chunk = dst blocks [0, SPLITB) of every core
RA = SPLITB * BLK  # 3072 rows per core in chunk A
RB = NSH - RA  # 3178 rows per core in chunk B
CCSB = 5  # fire AllGather-A after this superblock (covers blocks 0..23)


def _layout1(tiles):
    """Layer-1 static layout from per-(block,half) tile counts.

    Returns (TT, tile_base[NBLK][2], seg: {(sb,h): (tile0, ntiles)}).
    Data/program order: for sb, for half, for block in sb, k tiles.
    """
    tile_base = np.zeros((NBLK, 2), dtype=np.int64)
    seg = {}
    pos = 0
    for sb in range(NSB):
        blocks = range(sb * SBW, min((sb + 1) * SBW, NBLK))
        for h in (0, 1):
            seg_start = pos
            for b in blocks:
                tile_base[b][h] = pos
                pos += int(tiles[b][h])
            seg[(sb, h)] = (seg_start, pos - seg_start)
    return int(pos), tile_base, seg


def _rank_within_groups(gid):
    change = np.r_[True, gid[1:] != gid[:-1]]
    gstart = np.maximum.accumulate(np.where(change, np.arange(len(gid)), 0))
    return np.arange(len(gid)) - gstart


def _fill_meta_idx(core_s, slot, dval, nval, srcv, TT, S):
    """Build per-core meta [128, TT, 2] f32 and idx [128, S//16] i16 tables."""
    meta_np = np.zeros((NCORES, 128, TT, 2), dtype=np.float32)
    idx_np = np.zeros((NCORES, 128, S // 16), dtype=np.int16)
    for c in range(NCORES):
        m = core_s == c
        sl = slot[m]
        tt = sl // BLK
        pp = sl % BLK
        meta_np[c, pp, tt, 0] = dval[m]
        meta_np[c, pp, tt, 1] = nval[m]
        col = sl // 16
        row = sl % 16
        v = srcv[m]
        for g in range(8):  # replicate across the 8 gpsimd 16-partition groups
            idx_np[c, row + 16 * g, col] = v
    return meta_np, idx_np


def _prep(edge_index):
    src = np.asarray(edge_index[0], dtype=np.int64)
    dst = np.asarray(edge_index[1], dtype=np.int64)
    deg = (np.bincount(dst, minlength=N) + 1).astype(np.float64)
    dinv = (1.0 / np.sqrt(deg)).astype(np.float32)

    s_all = np.concatenate([src, np.arange(N, dtype=np.int64)])
    d_all = np.concatenate([dst, np.arange(N, dtype=np.int64)])
    norm_all = dinv[s_all] * dinv[d_all]
    core = d_all // NSH
    local = d_all % NSH
    block = local // BLK
    dstloc = (local % BLK).astype(np.float32)

    # ---- layer 1: halves (int16 src index into x halves), sb segments ----
    half = (s_all >= HALF).astype(np.int64)
    sbk = block // SBW
    cidx = (core * NBLK + block) * 2 + half
    cnt = np.bincount(cidx, minlength=NCORES * NBLK * 2).reshape(NCORES, NBLK, 2)
    tiles1 = ((cnt + BLK - 1) // BLK).max(axis=0)  # [NBLK, 2]
    TT1, tbase1, seg1 = _layout1(tiles1)
    S1 = TT1 * BLK

    order = np.lexsort((local, block, half, sbk, core))
    s_s = s_all[order]
    core_s = core[order]
    block_s = block[order]
    half_s = half[order]
    gid = (core_s * NBLK + block_s) * 2 + half_s
    rank = _rank_within_groups(gid)
    slot1 = tbase1[block_s, half_s] * BLK + rank
    src_loc = np.where(half_s == 0, s_s, s_s - HALF).astype(np.int16)
    meta1_np, idx1_np = _fill_meta_idx(
        core_s, slot1, dstloc[order], norm_all[order], src_loc, TT1, S1
    )

    # ---- layer 2: pair-packed split tables (A: src rows [0,RA) of each
    # core, B: the rest), phase-major so table-A work overlaps AllGather-B
    s_core = s_all // NSH
    s_r = s_all % NSH
    h2 = (s_r >= RA).astype(np.int64)
    cidx2 = (core * NBLK + block) * 2 + h2
    cnt2 = np.bincount(cidx2, minlength=NCORES * NBLK * 2).reshape(NCORES, NBLK, 2)
    tiles2 = ((cnt2 + BLK - 1) // BLK).max(axis=0)  # [NBLK, 2]
    TT2a = int(tiles2[:, 0].sum())
    TT2 = TT2a + int(tiles2[:, 1].sum())
    S2 = TT2 * BLK
    tbase2 = np.zeros((NBLK, 2), dtype=np.int64)
    tbase2[:, 0] = np.concatenate([[0], np.cumsum(tiles2[:, 0])[:-1]])
    tbase2[:, 1] = TT2a + np.concatenate([[0], np.cumsum(tiles2[:, 1])[:-1]])

    order2 = np.lexsort((local, block, h2, core))
    s2c = s_core[order2]
    s2r = s_r[order2]
    h2s = h2[order2]
    core2 = core[order2]
    block2 = block[order2]
    gid2 = (core2 * NBLK + block2) * 2 + h2s
    rank2 = _rank_within_groups(gid2)
    slot2 = tbase2[block2, h2s] * BLK + rank2
    srcp = np.where(
        h2s == 0, s2c * (RA // 2) + (s2r >> 1), s2c * (RB // 2) + ((s2r - RA) >> 1)
    ).astype(np.int16)
    dadj = dstloc[order2] + 128.0 * (s2r & 1)
    meta2_np, idx2_np = _fill_meta_idx(
        core2, slot2, dadj, norm_all[order2], srcp, TT2, S2
    )

    layout = (tiles1, tiles2)
    return layout, meta1_np, idx1_np, meta2_np, idx2_np


def _build(layout, ablate="full"):
    """Build the (single, SPMD) Bacc program for the given tile counts.

    ablate: "full" | "nocc" (local copy instead of AllGather) |
    "nogather" (memset instead of dma_gather) | "l1only" | "l2only".
    Non-"full" variants produce wrong results; timing probes only.
    """
    do_l1 = ablate != "l2only"
    do_l2 = ablate != "l1only"
    do_cc = ablate not in ("nocc", "l1only")
    do_gather = ablate != "nogather"
    tiles1, tiles2 = layout
    TT1, tbase1, seg1 = _layout1(tiles1)
    S1 = TT1 * BLK
    TT2a = int(tiles2[:, 0].sum())
    TT2 = TT2a + int(tiles2[:, 1].sum())
    S2 = TT2 * BLK
    tbase2 = np.zeros((NBLK, 2), dtype=np.int64)
    tbase2[:, 0] = np.concatenate([[0], np.cumsum(tiles2[:, 0])[:-1]])
    tbase2[:, 1] = TT2a + np.concatenate([[0], np.cumsum(tiles2[:, 1])[:-1]])
    f32 = mybir.dt.float32
    bf16 = mybir.dt.bfloat16
    i16 = mybir.dt.int16
    AF = mybir.ActivationFunctionType
    OP = mybir.AluOpType

    nc = bacc.Bacc(
        "TRN2",
        target_bir_lowering=False,
        debug=False,
        num_devices=NCORES,
        num_swdge_queues=4,
    )
    xin = nc.dram_tensor("xin", [N, FIN], bf16, kind="ExternalInput")
    w1 = nc.dram_tensor("w1", [FIN, HID], bf16, kind="ExternalInput")
    w2 = nc.dram_tensor("w2", [HID, FOUT], bf16, kind="ExternalInput")
    b1c = nc.dram_tensor("b1c", [HID, 1], f32, kind="ExternalInput")
    b2r = nc.dram_tensor("b2r", [128, FOUT], f32, kind="ExternalInput")
    iot = nc.dram_tensor("iot", [128, BLK], bf16, kind="ExternalInput")
    io2 = nc.dram_tensor("io2", [128, 2 * BLK], bf16, kind="ExternalInput")
    idn = nc.dram_tensor("idn", [128, 128], bf16, kind="ExternalInput")
    meta1 = nc.dram_tensor("meta1", [128, TT1, 2], f32, kind="ExternalInput")
    idx1t = nc.dram_tensor("idx1", [128, S1 // 16], i16, kind="ExternalInput")
    meta2 = nc.dram_tensor("meta2", [128, TT2, 2], f32, kind="ExternalInput")
    idx2t = nc.dram_tensor("idx2", [128, S2 // 16], i16, kind="ExternalInput")
    outp = nc.dram_tensor("out", [NSH, FOUT], f32, kind="ExternalOutput")

    with tile.TileContext(nc) as tc, ExitStack() as ctx:
        const = ctx.enter_context(tc.tile_pool(name="const", bufs=1))
        dram = ctx.enter_context(tc.tile_pool(name="dram", bufs=1, space="DRAM"))
        gpool = ctx.enter_context(tc.tile_pool(name="g", bufs=3))
        g2pool = ctx.enter_context(tc.tile_pool(name="g2", bufs=3))
        tpp = ctx.enter_context(tc.tile_pool(name="tp", bufs=12))
        evp = ctx.enter_context(tc.tile_pool(name="ev", bufs=4))
        accp = ctx.enter_context(tc.tile_pool(name="acc", bufs=1))
        psa = ctx.enter_context(tc.tile_pool(name="psa", bufs=3, space="PSUM"))
        psd = ctx.enter_context(tc.tile_pool(name="psd", bufs=1, space="PSUM"))
        pst = ctx.enter_context(tc.tile_pool(name="pst", bufs=1, space="PSUM"))
        pso = ctx.enter_context(tc.tile_pool(name="pso", bufs=1, space="PSUM"))
        psb = ctx.enter_context(tc.tile_pool(name="psb", bufs=2, space="PSUM"))

        def cload(ap, shape, dtype, tag):
            t = const.tile(shape, dtype, tag=tag)
            nc.sync.dma_start(t[:], ap)
            return t

        w1_sb = cload(w1[:, :], [FIN, HID], bf16, "w1")
        w2_sb = cload(w2[:, :], [HID, FOUT], bf16, "w2")
        b1_sb = cload(b1c[:, :], [HID, 1], f32, "b1")
        b2_sb = cload(b2r[:, :], [128, FOUT], f32, "b2")
        iota_sb = cload(iot[:, :], [128, BLK], bf16, "iota")
        io2_sb = cload(io2[:, :], [128, 2 * BLK], bf16, "io2")
        idn_sb = cload(idn[:, :], [128, 128], bf16, "idn")
        meta1_sb = cload(meta1[:, :, :], [128, TT1, 2], f32, "meta1")
        idx1_sb = cload(idx1t[:, :], [128, S1 // 16], i16, "idx1")
        meta2_sb = cload(meta2[:, :, :], [128, TT2, 2], f32, "meta2")
        idx2_sb = cload(idx2t[:, :], [128, S2 // 16], i16, "idx2")

        t2locA = dram.tile([RA, FOUT], bf16, tag="t2locA")
        t2locB = dram.tile([RB, FOUT], bf16, tag="t2locB")
        t2fA = dram.tile([NCORES * RA, FOUT], bf16, tag="t2fA", addr_space="Shared")
        t2fB = dram.tile([NCORES * RB, FOUT], bf16, tag="t2fB", addr_space="Shared")

        qctr = [0]  # round-robin gather queue assignment

        def next_q():
            q = qctr[0] % 4
            qctr[0] += 1
            return q

        def finalize1(b, ps):
            """ps [x=128, d=128] f32 -> h2 = relu((Ahat X W1)+b1) @ W2 -> t2loc."""
            ag = evp.tile([128, BLK], bf16, tag="ag")
            nc.scalar.activation(ag[:], ps[:], AF.Copy)
            hp = psd.tile([128, HID], f32, tag="hp")
            nc.tensor.matmul(hp[:], lhsT=ag[:], rhs=w1_sb[:], start=True, stop=True)
            hpe = evp.tile([128, HID], bf16, tag="hpe")
            nc.scalar.activation(hpe[:], hp[:], AF.Copy)
            pT = pst.tile([128, BLK], bf16, tag="pT")
            nc.tensor.transpose(pT[:], hpe[:], idn_sb[:])
            ev = evp.tile([128, BLK], bf16, tag="evt")
            nc.scalar.activation(ev[:], pT[:], AF.Relu, bias=b1_sb[:, 0:1])
            h2p = pso.tile([128, FOUT], f32, tag="h2p")
            nc.tensor.matmul(h2p[:], lhsT=ev[:], rhs=w2_sb[:], start=True, stop=True)
            h2 = evp.tile([128, FOUT], bf16, tag="h2")
            nc.scalar.activation(h2[:], h2p[:], AF.Copy)
            r0 = b * BLK
            r1 = min(NSH, r0 + BLK)
            if b < SPLITB:
                nc.sync.dma_start(t2locA[r0:r1, :], h2[0 : r1 - r0, :])
            else:
                nc.sync.dma_start(t2locB[r0 - RA : r1 - RA, :], h2[0 : r1 - r0, :])

        def ccgather(ins_ap, outs_ap):
            nc.gpsimd.collective_compute(
                "AllGather",
                mybir.AluOpType.bypass,
                replica_groups=[list(range(NCORES))],
                ins=[ins_ap],
                outs=[outs_ap],
            )

        # ---- layer 1: aggregate raw X rows, then per-block W1/relu/W2 ----
        for sb in range(NSB if do_l1 else 0):
            if sb == CCSB + 1 and do_cc:
                # chunk-A table exchange; overlaps the layer-1 tail
                ccgather(t2locA[:].opt(), t2fA[:].opt())
            blocks = list(range(sb * SBW, min((sb + 1) * SBW, NBLK)))
            gt = {}
            for h in (0, 1):
                t0, ntl = seg1[(sb, h)]
                if ntl == 0:
                    continue
                g = gpool.tile([128, ntl, FIN], bf16, tag=f"g{h}")
                view = xin[0:HALF, :] if h == 0 else xin[HALF : 2 * HALF, :]
                if not do_gather:
                    nc.vector.memset(g[:], 0)
                for q0 in range(0, ntl, GCH) if do_gather else ():
                    qn = min(GCH, ntl - q0)
                    c0 = (t0 + q0) * 8  # idx columns (tile*128/16)
                    nc.gpsimd.dma_gather(
                        out_ap=g[:, q0 : q0 + qn, :],
                        in_ap=view,
                        idxs_ap=idx1_sb[:, c0 : c0 + qn * 8],
                        num_idxs=qn * 128,
                        num_idxs_reg=qn * 128,
                        elem_size=FIN,
                        queue_num=next_q(),
                    )
                gt[h] = (g, t0)
            for b in blocks:
                ps = psa.tile([128, BLK], f32, tag="psa")
                for h in (0, 1):
                    if seg1[(sb, h)][1] == 0 or tiles1[b][h] == 0:
                        continue
                    g, seg_t0 = gt[h]
                    for k in range(int(tiles1[b][h])):
                        t = int(tbase1[b][h]) + k
                        first = (k == 0) and (h == 0 or tiles1[b][0] == 0)
                        last = (k == int(tiles1[b][h]) - 1) and (
                            h == 1 or tiles1[b][1] == 0
                        )
                        tp = tpp.tile([128, BLK], bf16, tag="tp")
                        nc.vector.tensor_scalar(
                            out=tp[:],
                            in0=iota_sb[:],
                            scalar1=meta1_sb[:, t, 0:1],
                            scalar2=meta1_sb[:, t, 1:2],
                            op0=OP.is_equal,
                            op1=OP.mult,
                        )
                        nc.tensor.matmul(
                            ps[:],
                            lhsT=g[:, t - seg_t0, :],
                            rhs=tp[:],
                            start=first,
                            stop=last,
                        )
                finalize1(b, ps)

        # pair views of the exchanged chunk tables: [rows/2, 128]
        t2pA = t2fA[:, :].rearrange("(a b) c -> a (b c)", b=2)
        t2pB = t2fB[:, :].rearrange("(a b) c -> a (b c)", b=2)
        if not do_cc and do_l1 and do_l2:
            nc.sync.dma_start(t2fA[0:RA, :], t2locA[:, :])
            nc.sync.dma_start(t2fB[0:RB, :], t2locB[:, :])

        # ---- layer 2: gather node pairs, 256-wide one-hot, two matmuls.
        # Phase 0 consumes table A (runs while AllGather-B is in flight,
        # accumulating per-block partials in SBUF); phase 1 consumes B.
        def writeout(b, oo):
            r0 = b * BLK
            r1 = min(NSH, r0 + BLK)
            nc.sync.dma_start(outp[r0:r1, :], oo[0 : r1 - r0, :])

        acc = {}
        for h in (0, 1) if do_l2 else ():
            if h == 1 and do_cc:
                ccgather(t2locB[:].opt(), t2fB[:].opt())
            view = t2pA if h == 0 else t2pB
            pt0 = 0 if h == 0 else TT2a
            pt1 = TT2a if h == 0 else TT2
            btile = np.repeat(np.arange(NBLK), tiles2[:, h])
            g2 = None
            ps2 = None
            for t in range(pt0, pt1):
                if (t - pt0) % GCH == 0:
                    qn = min(GCH, pt1 - t)
                    g2 = g2pool.tile([128, GCH, FIN], bf16, tag="g2")
                    if do_gather:
                        nc.gpsimd.dma_gather(
                            out_ap=g2[:, 0:qn, :],
                            in_ap=view,
                            idxs_ap=idx2_sb[:, t * 8 : (t + qn) * 8],
                            num_idxs=qn * 128,
                            num_idxs_reg=qn * 128,
                            elem_size=FIN,
                            queue_num=next_q(),
                        )
                    else:
                        nc.vector.memset(g2[:], 0)
                b = int(btile[t - pt0])
                k = t - int(tbase2[b][h])
                first = k == 0
                last = k == int(tiles2[b][h]) - 1
                if first:
                    ps2 = psb.tile([128, FOUT], f32, tag="psb")
                tp2 = tpp.tile([128, 2 * BLK], bf16, tag="tp2")
                nc.vector.tensor_scalar(
                    out=tp2[:],
                    in0=io2_sb[:],
                    scalar1=meta2_sb[:, t, 0:1],
                    scalar2=meta2_sb[:, t, 1:2],
                    op0=OP.is_equal,
                    op1=OP.mult,
                )
                j = (t - pt0) % GCH
                nc.tensor.matmul(
                    ps2[:],
                    lhsT=tp2[:, 0:BLK],
                    rhs=g2[:, j, 0:FOUT],
                    start=first,
                    stop=False,
                )
                nc.tensor.matmul(
                    ps2[:],
                    lhsT=tp2[:, BLK : 2 * BLK],
                    rhs=g2[:, j, FOUT:FIN],
                    start=False,
                    stop=last,
                )
                if not last:
                    continue
                if h == 0:
                    if tiles2[b][1] > 0:
                        ab = accp.tile([128, FOUT], f32, tag=f"acc{b}")
                        nc.vector.tensor_add(ab[:], ps2[:], b2_sb[:])
                        acc[b] = ab
                    else:
                        oo = evp.tile([128, FOUT], f32, tag="oo")
                        nc.vector.tensor_add(oo[:], ps2[:], b2_sb[:])
                        writeout(b, oo)
                else:
                    oo = evp.tile([128, FOUT], f32, tag="oo")
                    nc.vector.tensor_add(
                        oo[:], ps2[:], acc[b][:] if b in acc else b2_sb[:]
                    )
                    writeout(b, oo)

    nc.finalize()
    return nc


def _in_maps(x, W1, b1, W2, b2, prep):
    layout, meta1_np, idx1_np, meta2_np, idx2_np = prep
    xb = np.asarray(x, np.float32).astype(BF16)
    w1b = np.asarray(W1, np.float32).astype(BF16)
    w2b = np.asarray(W2, np.float32).astype(BF16)
    b1col = np.asarray(b1, np.float32).reshape(HID, 1).copy()
    b2rep = np.broadcast_to(np.asarray(b2, np.float32), (128, FOUT)).copy()
    iota = np.broadcast_to(np.arange(BLK, dtype=np.float32), (128, BLK)).astype(BF16)
    io2 = np.broadcast_to(np.arange(2 * BLK, dtype=np.float32), (128, 2 * BLK)).astype(
        BF16
    )
    idn = np.eye(128, dtype=np.float32).astype(BF16)
    shared = {
        "xin": np.ascontiguousarray(xb),
        "w1": w1b,
        "w2": w2b,
        "b1c": b1col,
        "b2r": b2rep,
        "iot": np.ascontiguousarray(iota),
        "io2": np.ascontiguousarray(io2),
        "idn": idn,
    }
    return [
        dict(
            shared,
            meta1=np.ascontiguousarray(meta1_np[c]),
            idx1=np.ascontiguousarray(idx1_np[c]),
            meta2=np.ascontiguousarray(meta2_np[c]),
            idx2=np.ascontiguousarray(idx2_np[c]),
        )
        for c in range(NCORES)
    ]


def kernel(x, edge_index, W1, b1, W2, b2):
    prep = _prep(edge_index)
    nc = _build(prep[0])
    in_maps = _in_maps(x, W1, b1, W2, b2, prep)
    res = run_bass_kernel_spmd(nc, in_maps, core_ids=list(range(NCORES)), trace=False)
    out = np.concatenate(
        [res.results[c]["out"].astype(np.float32) for c in range(NCORES)], axis=0
    )
    return out
